# revision 1
# baseline (speedup 1.0000x reference)
"""Trainium2 Bass kernel for nn_BnDCN_Context (maxpool + DCNv2 + BN/ReLU + GCNet + 1x1 fusion).

Sharding: 8 cores = 4 samples x 2 row-halves; each core owns 32 pooled rows
(2048 output pixels) of one sample, with a 5-row halo band for the deformable
gather. Two launches; the host only sums ~6KB of per-core partial statistics
between them (BN batch stats + GCNet softmax partials = the collective step).

Phase A: maxpool -> offset/mod conv -> deformable bilinear gather (dma_gather
         from a private pixel-major DRAM map, bf16) -> combine -> DCN matmul
         -> BN partial sums + GCNet attention partials.
Phase B: BN apply + ReLU, GCNet MLP + LayerNorm, 1x1 fusion with folded
         residual, output.
"""
import os
import numpy as np
import ml_dtypes

import concourse.bass as bass
import concourse.bacc as bacc
import concourse.tile as tile
from concourse import mybir
from concourse import library_config
from concourse.bass_utils import run_bass_kernel_spmd

F32 = mybir.dt.float32
BF16 = mybir.dt.bfloat16
I16 = mybir.dt.int16
I32 = mybir.dt.int32
ALU = mybir.AluOpType
AF = mybir.ActivationFunctionType
BF = ml_dtypes.bfloat16

B, C, HI, WI = 4, 256, 128, 128
H = W = 64
HP = WP = 66
OWN = 32
NPIX = OWN * W                 # 2048
BAND = 42                      # local map rows (own 32 + 5 halo each side)
OWN0 = 5                       # local map row of first own data row
MPIX = BAND * HP               # 2772
MCH = (MPIX + 127) // 128      # 22 map chunks
MAP_ROWS = 2816
QHI = float(BAND - 1)          # local row clip hi (41)
NTAP = 9
RR = C // 4                    # 64
N_TOT = float(B * H * W)       # 16384 (BN normalizer)
EPS = 1e-5

SIG = ((np.arange(128) % 16) * 8 + np.arange(128) // 16).astype(np.int64)


def build_phase_a():
    nc = bacc.Bacc("TRN2", target_bir_lowering=False)

    xin = nc.dram_tensor("xin", [2, 128, 84 * WI], BF16, kind="ExternalInput")
    p0xl8 = nc.dram_tensor("p0xl8", [128, 16 * NTAP], F32, kind="ExternalInput")
    p0yl8 = nc.dram_tensor("p0yl8", [128, 16 * NTAP], F32, kind="ExternalInput")
    p0xs = nc.dram_tensor("p0xs", [128, 16 * NTAP], F32, kind="ExternalInput")
    p0ys = nc.dram_tensor("p0ys", [128, 16 * NTAP], F32, kind="ExternalInput")
    ownm = nc.dram_tensor("ownm", [128, MCH], F32, kind="ExternalInput")
    cmb = nc.dram_tensor("cmb", [128, 1], F32, kind="ExternalInput")
    pmw = nc.dram_tensor("pmw", [2, 128, NTAP * 27], BF16, kind="ExternalInput")
    pmb = nc.dram_tensor("pmb", [1, 27], BF16, kind="ExternalInput")
    dcnw = nc.dram_tensor("dcnw", [2, 128, NTAP * C], BF16, kind="ExternalInput")
    dcnb = nc.dram_tensor("dcnb", [1, C], BF16, kind="ExternalInput")
    cmw = nc.dram_tensor("cmw", [2, 128], BF16, kind="ExternalInput")
    identb = nc.dram_tensor("identb", [128, 128], BF16, kind="ExternalInput")
    identf = nc.dram_tensor("identf", [128, 128], F32, kind="ExternalInput")

    y_out = nc.dram_tensor("y_out", [2, 128, NPIX], F32, kind="ExternalOutput")
    pooled_out = nc.dram_tensor("pooled_out", [2, 128, NPIX], BF16, kind="ExternalOutput")
    stats = nc.dram_tensor("stats", [1, 1032], F32, kind="ExternalOutput")

    mapd = nc.dram_tensor("mapd", [MAP_ROWS, C], BF16)
    wrapd = nc.dram_tensor("wrapd", [16, 4096], I16)

    with tile.TileContext(nc) as tc:
        with tc.tile_pool(name="singles", bufs=1) as singles, \
             tc.tile_pool(name="workp", bufs=int(os.environ.get("WB", "3"))) as workp, \
             tc.tile_pool(name="mapp", bufs=int(os.environ.get("MB", "4"))) as mapp, \
             tc.tile_pool(name="gpool", bufs=int(os.environ.get("GB", "2"))) as gpool, \
             tc.tile_pool(name="xop", bufs=int(os.environ.get("XB", "3"))) as xop, \
             tc.tile_pool(name="psA", bufs=1, space="PSUM") as psA, \
             tc.tile_pool(name="psXO", bufs=1, space="PSUM") as psXO, \
             tc.tile_pool(name="psY", bufs=1, space="PSUM") as psY, \
             tc.tile_pool(name="psCTX", bufs=1, space="PSUM") as psCTX:

            # ----- constants -----
            sb_p0xl8 = singles.tile([128, 16, NTAP], F32)
            sb_p0yl8 = singles.tile([128, 16, NTAP], F32)
            sb_p0xs = singles.tile([128, 16, NTAP], F32)
            sb_p0ys = singles.tile([128, 16, NTAP], F32)
            for t, d in ((sb_p0xl8, p0xl8), (sb_p0yl8, p0yl8), (sb_p0xs, p0xs), (sb_p0ys, p0ys)):
                nc.sync.dma_start(out=t, in_=d[:, :])
            sb_own = singles.tile([128, MCH], F32)
            nc.sync.dma_start(out=sb_own, in_=ownm[:, :])
            sb_cmb = singles.tile([128, 1], F32)
            nc.sync.dma_start(out=sb_cmb, in_=cmb[:, :])
            sb_pmw = singles.tile([128, 2, NTAP, 27], BF16)
            for ch in range(2):
                nc.sync.dma_start(out=sb_pmw[:, ch],
                                  in_=pmw[ch].rearrange("p (n o) -> p n o", n=NTAP))
            sb_pmb = singles.tile([1, 27], BF16)
            nc.sync.dma_start(out=sb_pmb, in_=pmb[:, :])
            sb_dcnw = singles.tile([128, 2, NTAP, C], BF16)
            for ch in range(2):
                nc.sync.dma_start(out=sb_dcnw[:, ch],
                                  in_=dcnw[ch].rearrange("p (n o) -> p n o", n=NTAP))
            sb_dcnb = singles.tile([1, C], BF16)
            nc.sync.dma_start(out=sb_dcnb, in_=dcnb[:, :])
            sb_cmw = singles.tile([128, 2], BF16)
            nc.sync.dma_start(out=sb_cmw, in_=cmw.rearrange("a p -> p a"))
            sb_idb = singles.tile([128, 128], BF16)
            nc.sync.dma_start(out=sb_idb, in_=identb[:, :])
            sb_idf = singles.tile([128, 128], F32)
            nc.sync.dma_start(out=sb_idf, in_=identf[:, :])
            sb_ones = singles.tile([1, 512], BF16)
            nc.vector.memset(sb_ones, 1.0)

            # ----- pooling into padded band map (channel-major bf16) -----
            band = [singles.tile([128, BAND, HP], BF16, tag=f"band{c_}", name=f"band{c_}") for c_ in range(2)]
            for ch in range(2):
                nc.vector.memset(band[ch], 0.0)
                for rc in range(6):  # 7 virtual pooled rows per chunk
                    raw = workp.tile([128, 14, WI], BF16, tag="raw")
                    nc.sync.dma_start(out=raw, in_=xin[ch, :, rc * 14 * WI:(rc + 1) * 14 * WI])
                    rowmax = workp.tile([128, 7, WI], BF16, tag="rowmax")
                    even = bass.AP(tensor=raw.tensor, offset=raw.offset,
                                   ap=[raw.ap[0], [2 * WI, 7], [1, WI]])
                    odd = bass.AP(tensor=raw.tensor, offset=raw.offset + WI,
                                  ap=[raw.ap[0], [2 * WI, 7], [1, WI]])
                    nc.vector.tensor_tensor(out=rowmax, in0=even, in1=odd, op=ALU.max)
                    ceven = bass.AP(tensor=rowmax.tensor, offset=rowmax.offset,
                                    ap=[rowmax.ap[0], [WI, 7], [2, W]])
                    codd = bass.AP(tensor=rowmax.tensor, offset=rowmax.offset + 1,
                                   ap=[rowmax.ap[0], [WI, 7], [2, W]])
                    dst = bass.AP(tensor=band[ch].tensor,
                                  offset=band[ch].offset + (rc * 7) * HP + 1,
                                  ap=[band[ch].ap[0], [HP, 7], [1, W]])
                    nc.vector.tensor_tensor(out=dst, in0=ceven, in1=codd, op=ALU.max)

            # ----- pooled own rows -> DRAM (phase B) -----
            for ch in range(2):
                src = bass.AP(tensor=band[ch].tensor,
                              offset=band[ch].offset + OWN0 * HP + 1,
                              ap=[band[ch].ap[0], [HP, OWN], [1, W]])
                nc.sync.dma_start(out=pooled_out[ch], in_=src)

            # ----- map transposes + GCNet attention partials -----
            bandf = [band[c_].rearrange("p a b -> p (a b)") for c_ in range(2)]
            ctx_ps = psCTX.tile([1, 257], F32)
            for m in range(MCH):
                valid = 128 if m < MCH - 1 else MPIX - 128 * (MCH - 1)
                # mask logit for these map pixels
                mk = psA.tile([128, 1], F32, tag="misc")
                for ch in range(2):
                    nc.tensor.matmul(mk[:valid], bandf[ch][:, m * 128: m * 128 + valid],
                                     sb_cmw[:, ch:ch + 1],
                                     start=(ch == 0), stop=(ch == 1))
                e_f = workp.tile([128, 1], F32, tag="e_f")
                nc.scalar.activation(out=e_f[:valid], in_=mk[:valid], func=AF.Exp,
                                     bias=sb_cmb[:valid], scale=1.0)
                e_b = workp.tile([128, 1], BF16, tag="e_b")
                nc.vector.tensor_tensor(out=e_b[:valid], in0=e_f[:valid],
                                        in1=sb_own[:valid, m:m + 1], op=ALU.mult)
                # pixel-major tile [pix, (c0|c1|ones)]
                xpm = mapp.tile([128, 257], BF16, tag="xpm")
                for ch in range(2):
                    tp = psA.tile([128, 128], BF16, tag="misc")
                    nc.tensor.transpose(tp[:valid], bandf[ch][:, m * 128: m * 128 + valid], sb_idb)
                    nc.scalar.copy(xpm[:valid, ch * 128:(ch + 1) * 128], tp[:valid])
                nc.vector.memset(xpm[:, 256:257], 1.0)
                nc.sync.dma_start(out=mapd[m * 128: m * 128 + valid, :], in_=xpm[:valid, 0:256])
                nc.tensor.matmul(ctx_ps, e_b[:valid], xpm[:valid],
                                 start=(m == 0), stop=(m == MCH - 1))
            zrow = workp.tile([1, 256], BF16, tag="zrow")
            nc.vector.memset(zrow, 0.0)
            nc.sync.dma_start(out=mapd[MPIX:MPIX + 1, :], in_=zrow)
            ctx_sb = workp.tile([1, 257], F32, tag="ctxsb")
            nc.vector.tensor_copy(ctx_sb, ctx_ps)
            nc.sync.dma_start(out=bass.AP(tensor=stats, offset=512, ap=[[1, 1], [1, 257]]),
                              in_=ctx_sb)

            # ----- offset/mod conv (27 ch) -----
            off_sb = singles.tile([27, NPIX], F32)
            for pt in range(4):
                ps = psA.tile([27, 512], F32, tag="misc")
                first = True
                for ch in range(2):
                    for n in range(NTAP):
                        dy, dx = n // 3, n % 3
                        rhs = bass.AP(tensor=band[ch].tensor,
                                      offset=band[ch].offset + (OWN0 - 1 + 8 * pt + dy) * HP + dx,
                                      ap=[band[ch].ap[0], [HP, 8], [1, W]])
                        nc.tensor.matmul(ps, sb_pmw[:, ch, n], rhs, start=first, stop=False)
                        first = False
                nc.tensor.matmul(ps, sb_pmb, sb_ones, start=False, stop=True)
                nc.scalar.copy(off_sb[:, pt * 512:(pt + 1) * 512], ps)

            # ----- off transposes: natural + sigma layouts -----
            offnat = singles.tile([128, 16, 27], F32)
            offsig = singles.tile([128, 16, 27], F32)
            off_sg = singles.tile([27, NPIX], F32)
            for t in range(16):
                srcp = bass.AP(tensor=off_sb.tensor, offset=off_sb.offset + t * 128,
                               ap=[off_sb.ap[0], [1, 8], [8, 16]])
                nc.vector.tensor_copy(off_sg[:, t * 128:(t + 1) * 128], srcp)
            for t in range(16):
                tpn = psA.tile([128, 27], F32, tag="misc")
                nc.tensor.transpose(tpn, off_sb[:, t * 128:(t + 1) * 128], sb_idf[0:27, 0:27])
                nc.vector.tensor_copy(offnat[:, t], tpn)
                tps = psA.tile([128, 27], F32, tag="misc")
                nc.tensor.transpose(tps, off_sg[:, t * 128:(t + 1) * 128], sb_idf[0:27, 0:27])
                nc.vector.tensor_copy(offsig[:, t], tps)

            # ----- index math (natural layout) -----
            shp = [128, 16, NTAP]
            fxm8 = workp.tile(shp, F32, tag="im1")
            fym8 = workp.tile(shp, F32, tag="im2")
            ii = workp.tile(shp, I32, tag="imi")
            for (dst, sl) in ((fxm8, 0), (fym8, NTAP)):
                nc.vector.tensor_scalar_add(dst, offnat[:, :, sl:sl + NTAP], 7.5)
                nc.vector.tensor_copy(ii, dst)
                nc.vector.tensor_copy(dst, ii)
            qlx = workp.tile(shp, F32, tag="im3")
            qly = workp.tile(shp, F32, tag="im4")
            nc.vector.tensor_tensor(out=qlx, in0=fxm8, in1=sb_p0xl8, op=ALU.add)
            nc.vector.tensor_scalar(out=qlx, in0=qlx, scalar1=0.0, scalar2=QHI,
                                    op0=ALU.max, op1=ALU.min)
            nc.vector.tensor_tensor(out=qly, in0=fym8, in1=sb_p0yl8, op=ALU.add)
            nc.vector.tensor_scalar(out=qly, in0=qly, scalar1=0.0, scalar2=65.0,
                                    op0=ALU.max, op1=ALU.min)
            qrx = workp.tile(shp, F32, tag="im5")
            nc.vector.tensor_scalar(out=qrx, in0=qlx, scalar1=1.0, scalar2=QHI,
                                    op0=ALU.add, op1=ALU.min)
            # idx staging S [128, 512] f32, layout v = pair*256 + g*128 + n*8 + tl
            S = singles.tile([128, 512], F32)
            nc.vector.memset(S, 0.0)
            for pair, rows in ((0, qlx), (1, qrx)):
                for g in range(2):
                    src0 = bass.AP(tensor=rows.tensor, offset=rows.offset + g * 72,
                                   ap=[rows.ap[0], [9, 8], [1, NTAP]])
                    src1 = bass.AP(tensor=qly.tensor, offset=qly.offset + g * 72,
                                   ap=[qly.ap[0], [9, 8], [1, NTAP]])
                    dstS = bass.AP(tensor=S.tensor, offset=S.offset + pair * 256 + g * 128,
                                   ap=[S.ap[0], [1, 8], [8, NTAP]])
                    nc.vector.scalar_tensor_tensor(out=dstS, in0=src0, scalar=66.0, in1=src1,
                                                   op0=ALU.mult, op1=ALU.add)
            # S -> T -> wrapped dram -> idxw (replicated)
            for ck in range(4):
                tps = psA.tile([128, 128], F32, tag="misc")
                nc.tensor.transpose(tps, S[:, ck * 128:(ck + 1) * 128], sb_idf)
                ti = workp.tile([128, 128], I16, tag="Ti")
                nc.vector.tensor_copy(ti, tps)
                dst = bass.AP(tensor=wrapd, offset=ck * 1024,
                              ap=[[8, 128], [4096, 16], [1, 8]])
                src = bass.AP(tensor=ti.tensor, offset=ti.offset,
                              ap=[ti.ap[0], [8, 16], [1, 8]])
                nc.sync.dma_start(out=dst, in_=src)
            idxw = singles.tile([128, 4096], I16)
            nc.sync.dma_start(out=idxw[0:16, :], in_=wrapd[:, :])
            for r in range(1, 8):
                nc.sync.dma_start(out=idxw[16 * r:16 * (r + 1), :], in_=idxw[0:16, :])

            # ----- weight math (sigma layout) -----
            fxs = workp.tile(shp, F32, tag="wm1")
            fys = workp.tile(shp, F32, tag="wm2")
            iis = workp.tile(shp, I32, tag="wmi")
            for (dst, sl) in ((fxs, 0), (fys, NTAP)):
                nc.vector.tensor_scalar_add(dst, offsig[:, :, sl:sl + NTAP], 7.5)
                nc.vector.tensor_copy(iis, dst)
                nc.vector.tensor_copy(dst, iis)
                nc.vector.tensor_scalar_add(dst, dst, -8.0)   # floor(off)
            pxc = workp.tile(shp, F32, tag="wm3")
            pyc = workp.tile(shp, F32, tag="wm4")
            nc.vector.tensor_tensor(out=pxc, in0=offsig[:, :, 0:NTAP], in1=sb_p0xs, op=ALU.add)
            nc.vector.tensor_scalar(out=pxc, in0=pxc, scalar1=0.0, scalar2=65.0,
                                    op0=ALU.max, op1=ALU.min)
            nc.vector.tensor_tensor(out=pyc, in0=offsig[:, :, NTAP:2 * NTAP], in1=sb_p0ys, op=ALU.add)
            nc.vector.tensor_scalar(out=pyc, in0=pyc, scalar1=0.0, scalar2=65.0,
                                    op0=ALU.max, op1=ALU.min)
            qlxg = workp.tile(shp, F32, tag="wm5")
            qlyg = workp.tile(shp, F32, tag="wm6")
            nc.vector.tensor_tensor(out=qlxg, in0=fxs, in1=sb_p0xs, op=ALU.add)
            nc.vector.tensor_scalar(out=qlxg, in0=qlxg, scalar1=0.0, scalar2=65.0,
                                    op0=ALU.max, op1=ALU.min)
            nc.vector.tensor_tensor(out=qlyg, in0=fys, in1=sb_p0ys, op=ALU.add)
            nc.vector.tensor_scalar(out=qlyg, in0=qlyg, scalar1=0.0, scalar2=65.0,
                                    op0=ALU.max, op1=ALU.min)
            qrxg = workp.tile(shp, F32, tag="wm7")
            qryg = workp.tile(shp, F32, tag="wm8")
            nc.vector.tensor_scalar(out=qrxg, in0=qlxg, scalar1=1.0, scalar2=65.0,
                                    op0=ALU.add, op1=ALU.min)
            nc.vector.tensor_scalar(out=qryg, in0=qlyg, scalar1=1.0, scalar2=65.0,
                                    op0=ALU.add, op1=ALU.min)
            wxl = workp.tile(shp, F32, tag="wm9")
            wyl = workp.tile(shp, F32, tag="wm10")
            wxr = workp.tile(shp, F32, tag="wm11")
            wyr = workp.tile(shp, F32, tag="wm12")
            nc.vector.scalar_tensor_tensor(out=wxl, in0=qlxg, scalar=1.0, in1=pxc,
                                           op0=ALU.add, op1=ALU.subtract)
            nc.vector.scalar_tensor_tensor(out=wyl, in0=qlyg, scalar=1.0, in1=pyc,
                                           op0=ALU.add, op1=ALU.subtract)
            nc.vector.scalar_tensor_tensor(out=wxr, in0=qrxg, scalar=-1.0, in1=pxc,
                                           op0=ALU.mult, op1=ALU.add)
            nc.vector.tensor_scalar_add(wxr, wxr, 1.0)
            nc.vector.scalar_tensor_tensor(out=wyr, in0=qryg, scalar=-1.0, in1=pyc,
                                           op0=ALU.mult, op1=ALU.add)
            nc.vector.tensor_scalar_add(wyr, wyr, 1.0)
            modv = workp.tile(shp, F32, tag="wm13")
            nc.scalar.activation(out=modv, in_=offsig[:, :, 2 * NTAP:3 * NTAP],
                                 func=AF.Sigmoid, bias=0.0, scale=1.0)
            nc.vector.tensor_tensor(out=wxl, in0=wxl, in1=modv, op=ALU.mult)
            nc.vector.tensor_tensor(out=wxr, in0=wxr, in1=modv, op=ALU.mult)
            wA = singles.tile(shp, F32)
            wB = singles.tile(shp, F32)
            wC = singles.tile(shp, F32)
            wD = singles.tile(shp, F32)
            nc.vector.tensor_tensor(out=wA, in0=wxl, in1=wyl, op=ALU.mult)
            nc.vector.tensor_tensor(out=wB, in0=wxl, in1=wyr, op=ALU.mult)
            nc.vector.tensor_tensor(out=wC, in0=wxr, in1=wyl, op=ALU.mult)
            nc.vector.tensor_tensor(out=wD, in0=wxr, in1=wyr, op=ALU.mult)

            # ----- gather / combine / transpose / DCN matmul -----
            y_sb = [singles.tile([128, NPIX], F32, tag=f"ysb{c_}", name=f"ysb{c_}") for c_ in range(2)]
            map_ap = bass.AP(tensor=mapd, offset=0, ap=[[256, MAP_ROWS - 2], [1, 512]])
            for g in range(2):
                yps = [psY.tile([128, 512], F32, tag=f"yps{h}{o}", name=f"yps{h}{o}")
                       for h in range(2) for o in range(2)]
                for n in range(NTAP):
                    G = []
                    for pair in range(2):
                        gt = gpool.tile([128, 8, 512], BF16, tag=f"G{pair}")
                        blk = (pair * 2 + g) * 16 + n
                        nc.gpsimd.dma_gather(
                            out_ap=gt[:, :, :], in_ap=map_ap,
                            idxs_ap=idxw[:, blk * 64:(blk + 1) * 64],
                            num_idxs=1024, num_idxs_reg=1024,
                            elem_size=512, elem_step=256)
                        G.append(gt)
                    for h in range(2):
                        xoc = [psXO.tile([128, 512], BF16, tag=f"xoc{c_}", name=f"xoc{c_}") for c_ in range(2)]
                        for tl4 in range(4):
                            tl = h * 4 + tl4
                            t_abs = g * 8 + tl
                            xo = xop.tile([128, 256], BF16, tag="xo")
                            nc.vector.tensor_scalar_mul(xo, G[0][:, tl, 0:256],
                                                        wA[:, t_abs, n:n + 1])
                            nc.vector.scalar_tensor_tensor(out=xo, in0=G[0][:, tl, 256:512],
                                                           scalar=wB[:, t_abs, n:n + 1], in1=xo,
                                                           op0=ALU.mult, op1=ALU.add)
                            nc.vector.scalar_tensor_tensor(out=xo, in0=G[1][:, tl, 0:256],
                                                           scalar=wC[:, t_abs, n:n + 1], in1=xo,
                                                           op0=ALU.mult, op1=ALU.add)
                            nc.vector.scalar_tensor_tensor(out=xo, in0=G[1][:, tl, 256:512],
                                                           scalar=wD[:, t_abs, n:n + 1], in1=xo,
                                                           op0=ALU.mult, op1=ALU.add)
                            for ch in range(2):
                                nc.tensor.transpose(xoc[ch][:, tl4 * 128:(tl4 + 1) * 128],
                                                    xo[:, ch * 128:(ch + 1) * 128], sb_idb)
                        xos = [xop.tile([128, 512], BF16, tag=f"xos{c_}", name=f"xos{c_}") for c_ in range(2)]
                        for ch in range(2):
                            nc.scalar.copy(xos[ch], xoc[ch])
                        for ch in range(2):
                            for o in range(2):
                                nc.tensor.matmul(yps[h * 2 + o],
                                                 sb_dcnw[:, ch, n, o * 128:(o + 1) * 128],
                                                 xos[ch],
                                                 start=(n == 0 and ch == 0), stop=False)
                for h in range(2):
                    for o in range(2):
                        nc.tensor.matmul(yps[h * 2 + o], sb_dcnb[:, o * 128:(o + 1) * 128],
                                         sb_ones, start=False, stop=True)
                        # un-permute sigma on the copy out (per 128-pixel block)
                        for tl4 in range(4):
                            dsty = bass.AP(tensor=y_sb[o].tensor,
                                           offset=y_sb[o].offset + (g * 2 + h) * 512 + tl4 * 128,
                                           ap=[y_sb[o].ap[0], [1, 8], [8, 16]])
                            srcy = bass.AP(tensor=yps[h * 2 + o].tensor,
                                           offset=yps[h * 2 + o].offset + tl4 * 128,
                                           ap=[yps[h * 2 + o].ap[0], [16, 8], [1, 16]])
                            nc.scalar.copy(dsty, srcy)

            # ----- BN partial sums + outputs -----
            scratch = workp.tile([128, NPIX], BF16, tag="scr")
            s1 = workp.tile([128, 1], F32, tag="s1")
            s2 = workp.tile([128, 1], F32, tag="s2")
            for ch in range(2):
                nc.scalar.activation(out=scratch, in_=y_sb[ch], func=AF.Copy,
                                     accum_out=s1)
                nc.scalar.activation(out=scratch, in_=y_sb[ch], func=AF.Square,
                                     accum_out=s2)
                nc.sync.dma_start(out=bass.AP(tensor=stats, offset=ch * 128, ap=[[1, 128], [1, 1]]),
                                  in_=s1)
                nc.sync.dma_start(out=bass.AP(tensor=stats, offset=256 + ch * 128, ap=[[1, 128], [1, 1]]),
                                  in_=s2)
                nc.sync.dma_start(out=y_out[ch], in_=y_sb[ch])
    nc.compile()
    return nc


def build_phase_b():
    nc = bacc.Bacc("TRN2", target_bir_lowering=False)
    y_in = nc.dram_tensor("y_in", [2, 128, NPIX], F32, kind="ExternalInput")
    pooled_in = nc.dram_tensor("pooled_in", [2, 128, NPIX], BF16, kind="ExternalInput")
    bnsum = nc.dram_tensor("bnsum", [2, 128, 1], F32, kind="ExternalInput")
    bnsq = nc.dram_tensor("bnsq", [2, 128, 1], F32, kind="ExternalInput")
    ctxv = nc.dram_tensor("ctxv", [2, 128, 1], F32, kind="ExternalInput")
    bng = nc.dram_tensor("bng", [2, 128, 1], F32, kind="ExternalInput")
    bnb = nc.dram_tensor("bnb", [2, 128, 1], F32, kind="ExternalInput")
    fb = nc.dram_tensor("fb", [2, 128, 1], F32, kind="ExternalInput")
    c2b = nc.dram_tensor("c2b", [2, 128, 1], F32, kind="ExternalInput")
    c1wT = nc.dram_tensor("c1wT", [2, 128, RR], BF16, kind="ExternalInput")
    c1b = nc.dram_tensor("c1b", [RR, 1], F32, kind="ExternalInput")
    lng = nc.dram_tensor("lng", [1, RR], F32, kind="ExternalInput")
    lnb = nc.dram_tensor("lnb", [1, RR], F32, kind="ExternalInput")
    c2wT = nc.dram_tensor("c2wT", [RR, C], BF16, kind="ExternalInput")
    fwT = nc.dram_tensor("fwT", [128, 8, 128], BF16, kind="ExternalInput")
    identb = nc.dram_tensor("identb", [128, 128], BF16, kind="ExternalInput")
    identf = nc.dram_tensor("identf", [128, 128], F32, kind="ExternalInput")

    outh = nc.dram_tensor("outh", [2, 128, NPIX], F32, kind="ExternalOutput")

    with tile.TileContext(nc) as tc:
        with tc.tile_pool(name="singles", bufs=1) as singles, \
             tc.tile_pool(name="workp", bufs=2) as workp, \
             tc.tile_pool(name="ps", bufs=1, space="PSUM") as ps, \
             tc.tile_pool(name="psf", bufs=4, space="PSUM") as psf:
            ysb = [singles.tile([128, NPIX], F32, tag=f"y{c_}", name=f"yl{c_}") for c_ in range(2)]
            psb = [singles.tile([128, NPIX], BF16, tag=f"p{c_}", name=f"pl{c_}") for c_ in range(2)]
            for ch in range(2):
                nc.sync.dma_start(out=ysb[ch], in_=y_in[ch])
                nc.sync.dma_start(out=psb[ch], in_=pooled_in[ch])
            ld = {}
            for name, d, shp, dt in (("bnsum", bnsum, [128, 1], F32), ("bnsq", bnsq, [128, 1], F32),
                                     ("ctxv", ctxv, [128, 1], F32), ("bng", bng, [128, 1], F32),
                                     ("bnb", bnb, [128, 1], F32), ("fb", fb, [128, 1], F32),
                                     ("c2b", c2b, [128, 1], F32)):
                ld[name] = [singles.tile(shp, dt, tag=f"{name}{c_}", name=f"ld_{name}{c_}") for c_ in range(2)]
                for ch in range(2):
                    nc.sync.dma_start(out=ld[name][ch], in_=d[ch])
            sb_c1w = singles.tile([128, 2, RR], BF16)
            for ch in range(2):
                nc.sync.dma_start(out=sb_c1w[:, ch], in_=c1wT[ch])
            sb_c1b = singles.tile([RR, 1], F32)
            nc.sync.dma_start(out=sb_c1b, in_=c1b[:, :])
            sb_lng = singles.tile([1, RR], F32)
            nc.sync.dma_start(out=sb_lng, in_=lng[:, :])
            sb_lnb = singles.tile([1, RR], F32)
            nc.sync.dma_start(out=sb_lnb, in_=lnb[:, :])
            sb_c2w = singles.tile([RR, C], BF16)
            nc.sync.dma_start(out=sb_c2w, in_=c2wT[:, :])
            sb_fw = singles.tile([128, 8, 128], BF16)
            nc.sync.dma_start(out=sb_fw, in_=fwT[:, :])
            sb_idb = singles.tile([128, 128], BF16)
            nc.sync.dma_start(out=sb_idb, in_=identb[:, :])
            sb_idf = singles.tile([128, 128], F32)
            nc.sync.dma_start(out=sb_idf, in_=identf[:, :])
            epsv = singles.tile([128, 1], F32)
            nc.vector.memset(epsv, EPS)

            # BN scale/shift
            ybn = [singles.tile([128, NPIX], BF16, tag=f"ybn{c_}", name=f"ybn{c_}") for c_ in range(2)]
            zb = [singles.tile([128, NPIX], BF16, tag=f"z{c_}", name=f"zb{c_}") for c_ in range(2)]
            biasF = [workp.tile([128, 1], F32, tag=f"bf{c_}", name=f"biasF{c_}") for c_ in range(2)]
            for ch in range(2):
                mu = workp.tile([128, 1], F32, tag="mu")
                nc.vector.tensor_scalar_mul(mu, ld["bnsum"][ch], 1.0 / N_TOT)
                s2n = workp.tile([128, 1], F32, tag="s2n")
                nc.vector.tensor_scalar_mul(s2n, ld["bnsq"][ch], 1.0 / N_TOT)
                negmu = workp.tile([128, 1], F32, tag="negmu")
                nc.vector.tensor_scalar_mul(negmu, mu, -1.0)
                var = workp.tile([128, 1], F32, tag="var")
                nc.vector.scalar_tensor_tensor(out=var, in0=mu, scalar=negmu, in1=s2n,
                                               op0=ALU.mult, op1=ALU.add)
                std = workp.tile([128, 1], F32, tag="std")
                nc.scalar.activation(out=std, in_=var, func=AF.Sqrt, bias=epsv, scale=1.0)
                rstd = workp.tile([128, 1], F32, tag="rstd")
                nc.vector.reciprocal(rstd, std)
                scale = workp.tile([128, 1], F32, tag="scale")
                nc.vector.tensor_tensor(out=scale, in0=ld["bng"][ch], in1=rstd, op=ALU.mult)
                shift = workp.tile([128, 1], F32, tag="shift")
                nc.vector.scalar_tensor_tensor(out=shift, in0=scale, scalar=negmu,
                                               in1=ld["bnb"][ch], op0=ALU.mult, op1=ALU.add)
                nc.scalar.activation(out=ybn[ch], in_=ysb[ch], func=AF.Relu,
                                     bias=shift, scale=scale)

            # GCNet MLP
            ctxb = workp.tile([128, 2], BF16, tag="ctxb")
            for ch in range(2):
                nc.vector.tensor_copy(ctxb[:, ch:ch + 1], ld["ctxv"][ch])
            t1p = ps.tile([RR, 1], F32)
            for ch in range(2):
                nc.tensor.matmul(t1p, sb_c1w[:, ch], ctxb[:, ch:ch + 1],
                                 start=(ch == 0), stop=(ch == 1))
            t1s = workp.tile([RR, 1], F32, tag="t1s")
            nc.vector.tensor_tensor(out=t1s, in0=t1p, in1=sb_c1b, op=ALU.add)
            t1tp = ps.tile([1, RR], F32)
            nc.tensor.transpose(t1tp, t1s, sb_idf[0:RR, 0:RR])
            t1t = workp.tile([1, RR], F32, tag="t1t")
            nc.vector.tensor_copy(t1t, t1tp)
            m1 = workp.tile([1, 1], F32, tag="m1")
            nc.vector.tensor_reduce(m1, t1t, axis=mybir.AxisListType.X, op=ALU.add)
            nc.vector.tensor_scalar_mul(m1, m1, -1.0 / RR)   # -mean
            cen = workp.tile([1, RR], F32, tag="cen")
            nc.vector.tensor_scalar_add(cen, t1t, m1)
            sq = workp.tile([1, RR], F32, tag="sq")
            v1 = workp.tile([1, 1], F32, tag="v1")
            nc.vector.scalar_tensor_tensor(out=sq, in0=cen, scalar=1.0, in1=cen,
                                           op0=ALU.mult, op1=ALU.mult, accum_out=v1)
            nc.vector.tensor_scalar_mul(v1, v1, 1.0 / RR)
            nc.scalar.activation(out=v1, in_=v1, func=AF.Sqrt, bias=epsv[0:1], scale=1.0)
            nc.vector.reciprocal(v1, v1)
            tn = workp.tile([1, RR], F32, tag="tn")
            nc.vector.tensor_scalar_mul(tn, cen, v1)
            nc.vector.tensor_tensor(out=tn, in0=tn, in1=sb_lng, op=ALU.mult)
            nc.vector.tensor_tensor(out=tn, in0=tn, in1=sb_lnb, op=ALU.add)
            tr = workp.tile([1, RR], BF16, tag="tr")
            nc.scalar.activation(out=tr, in_=tn, func=AF.Relu, bias=0.0, scale=1.0)
            trtp = ps.tile([RR, 1], BF16)
            nc.tensor.transpose(trtp, tr, sb_idb[0:1, 0:1])
            trt = workp.tile([RR, 1], BF16, tag="trt")
            nc.vector.tensor_copy(trt, trtp)
            for ch in range(2):
                tp2 = ps.tile([128, 1], F32, tag="tp2")
                nc.tensor.matmul(tp2, sb_c2w[:, ch * 128:(ch + 1) * 128], trt,
                                 start=True, stop=True)
                tv = workp.tile([128, 1], F32, tag="tv")
                nc.vector.tensor_tensor(out=tv, in0=tp2, in1=ld["c2b"][ch], op=ALU.add)
                nc.vector.tensor_tensor(out=biasF[ch], in0=ld["fb"][ch], in1=tv, op=ALU.subtract)
                nc.vector.tensor_scalar_add(zb[ch], psb[ch], tv)

            # fusion
            outsb = [singles.tile([128, NPIX], F32, tag=f"o{c_}", name=f"outsb{c_}") for c_ in range(2)]
            rhs = [ybn[0], ybn[1], zb[0], zb[1]]
            for o in range(2):
                for pt in range(4):
                    pf = psf.tile([128, 512], F32, tag="pf")
                    for k in range(4):
                        nc.tensor.matmul(pf, sb_fw[:, k * 2 + o],
                                         rhs[k][:, pt * 512:(pt + 1) * 512],
                                         start=(k == 0), stop=(k == 3))
                    nc.scalar.activation(out=outsb[o][:, pt * 512:(pt + 1) * 512], in_=pf,
                                         func=AF.Identity, bias=biasF[o], scale=1.0)
                nc.sync.dma_start(out=outh[o], in_=outsb[o])
    nc.compile()
    return nc


# ---------------- host side ----------------
_CACHE = {}
EXEC_NS = []


def _run(nc, in_maps):
    if os.environ.get("KERNEL_SIM"):
        from concourse.bass_interp import CoreSim
        outs = []
        for i, im in enumerate(in_maps):
            sim = CoreSim(nc, require_finite=False, require_nnan=False)
            for k, v in im.items():
                sim.tensor(k)[:] = v
            sim.simulate(check_with_hw=False)
            out_allocs = {a.memorylocations[0].name: list(a.tensor_shape)
                          for a in nc.m.functions[0].allocations
                          if getattr(a, "kind", None) == "ExternalOutput"}
            outs.append({k: np.array(sim.mem_tensor(k)).reshape(shp)
                         for k, shp in out_allocs.items()})
            print(f"  sim core {i} done")
        return outs
    res = run_bass_kernel_spmd(nc, in_maps, core_ids=list(range(8)))
    if res.exec_time_ns is not None:
        EXEC_NS.append(res.exec_time_ns)
    return res.results


def _consts():
    if "c" in _CACHE:
        return _CACHE["c"]
    rng3 = np.arange(-1, 2)
    pnx = np.repeat(rng3, 3).astype(np.float32)   # tap n = (dy+1)*3+(dx+1)
    pny = np.tile(rng3, 3).astype(np.float32)
    p = np.arange(128)
    t = np.arange(16)
    s_nat = t[None, :] * 128 + p[:, None]          # [128,16]
    s_sig = t[None, :] * 128 + SIG[p][:, None]
    consts = {}
    for hh in range(2):
        g0 = 1 + 32 * hh
        r_nat = s_nat // 64
        c_nat = s_nat % 64
        r_sig = s_sig // 64
        c_sig = s_sig % 64
        consts[hh] = dict(
            p0xl8=(OWN0 + r_nat[:, :, None] + pnx[None, None, :] - 8.0).astype(np.float32).reshape(128, -1),
            p0yl8=(c_nat[:, :, None] + 1 + pny[None, None, :] - 8.0).astype(np.float32).reshape(128, -1),
            p0xs=(g0 + r_sig[:, :, None] + pnx[None, None, :]).astype(np.float32).reshape(128, -1),
            p0ys=(c_sig[:, :, None] + 1 + pny[None, None, :]).astype(np.float32).reshape(128, -1),
        )
    mp = np.arange(MCH * 128)
    mrow, mcol = mp // HP, mp % HP
    own = ((mrow >= OWN0) & (mrow < OWN0 + OWN) & (mcol >= 1) & (mcol < 65) & (mp < MPIX))
    ownm = own.astype(np.float32).reshape(MCH, 128).T.copy()   # [128, MCH]
    identb = np.eye(128, dtype=BF)
    identf = np.eye(128, dtype=np.float32)
    _CACHE["c"] = (consts, ownm, identb, identf)
    return _CACHE["c"]


def kernel(x, p_w, p_b, m_w, m_b, dcn_w, dcn_b, bn_g, bn_b,
           cm_w, cm_b, c1_w, c1_b, ln_g, ln_b, c2_w, c2_b, f_w, f_b):
    x = np.asarray(x, np.float32)
    consts, ownm, identb, identf = _consts()

    # weights prep
    pm = np.concatenate([np.asarray(p_w), np.asarray(m_w)], 0).astype(np.float32)  # [27,256,3,3]
    pmw = np.zeros((2, 128, NTAP * 27), BF)
    for ch in range(2):
        for n in range(NTAP):
            pmw[ch, :, n * 27:(n + 1) * 27] = pm[:, ch * 128:(ch + 1) * 128, n // 3, n % 3].T.astype(BF)
    pmb_h = np.concatenate([np.asarray(p_b), np.asarray(m_b)]).astype(BF)[None, :]
    dw = np.asarray(dcn_w, np.float32).reshape(C, C, NTAP)
    dcnw_h = np.zeros((2, 128, NTAP * C), BF)
    for ch in range(2):
        for n in range(NTAP):
            dcnw_h[ch, :, n * C:(n + 1) * C] = dw[:, ch * 128:(ch + 1) * 128, n].T.astype(BF)
    dcnb_h = np.asarray(dcn_b, np.float32).astype(BF)[None, :]
    cmw_h = np.asarray(cm_w, np.float32).reshape(C).astype(BF).reshape(2, 128)
    cmb_h = np.full((128, 1), float(np.asarray(cm_b).reshape(-1)[0]), np.float32)
    fw2 = np.asarray(f_w, np.float32).reshape(C, 2 * C)
    fwp = fw2.copy()
    fwp[:, C:] += np.eye(C, dtype=np.float32)
    fwT_h = np.zeros((128, 8, 128), BF)
    for kc in range(4):
        for oc in range(2):
            fwT_h[:, kc * 2 + oc, :] = fwp[oc * 128:(oc + 1) * 128, kc * 128:(kc + 1) * 128].T.astype(BF)
    c1w2 = np.asarray(c1_w, np.float32).reshape(RR, C)
    c1wT_h = np.stack([c1w2[:, ch * 128:(ch + 1) * 128].T.astype(BF) for ch in range(2)])
    c2w2 = np.asarray(c2_w, np.float32).reshape(C, RR)
    c2wT_h = c2w2.T.astype(BF)                      # [RR, C]
    two = lambda v: np.asarray(v, np.float32).reshape(2, 128, 1)
    bng_h, bnb_h, fb_h, c2b_h = two(bn_g), two(bn_b), two(f_b), two(c2_b)
    c1b_h = np.asarray(c1_b, np.float32).reshape(RR, 1)
    lng_h = np.asarray(ln_g, np.float32).reshape(1, RR)
    lnb_h = np.asarray(ln_b, np.float32).reshape(1, RR)

    xbf = x.astype(BF)
    in_maps_a = []
    for i in range(8):
        s, hh = i // 2, i % 2
        g0 = 1 + 32 * hh
        xin = np.zeros((2, 128, 84, WI), BF)
        for l in range(BAND):
            pr = g0 - 6 + l
            if 0 <= pr < 64:
                xin[:, :, 2 * l:2 * l + 2, :] = xbf[s].reshape(2, 128, HI, WI)[:, :, 2 * pr:2 * pr + 2, :]
        cc = consts[hh]
        in_maps_a.append(dict(
            xin=xin.reshape(2, 128, 84 * WI),
            p0xl8=cc["p0xl8"], p0yl8=cc["p0yl8"], p0xs=cc["p0xs"], p0ys=cc["p0ys"],
            ownm=ownm, cmb=cmb_h, pmw=pmw, pmb=pmb_h, dcnw=dcnw_h, dcnb=dcnb_h,
            cmw=cmw_h, identb=identb, identf=identf,
        ))

    if "nc_a" not in _CACHE:
        _CACHE["nc_a"] = build_phase_a()
        _CACHE["nc_b"] = build_phase_b()
    ra = _run(_CACHE["nc_a"], in_maps_a)

    st = np.stack([ra[i]["stats"][0] for i in range(8)])   # [8, 1032]
    bnsum_tot = st[:, 0:256].sum(0).reshape(2, 128, 1).astype(np.float32)
    bnsq_tot = st[:, 256:512].sum(0).reshape(2, 128, 1).astype(np.float32)
    ctx_all = []
    for s in range(4):
        p1 = st[2 * s, 512:768] + st[2 * s + 1, 512:768]
        z = st[2 * s, 768] + st[2 * s + 1, 768]
        ctx_all.append((p1 / z).reshape(2, 128, 1).astype(np.float32))

    in_maps_b = []
    for i in range(8):
        s = i // 2
        in_maps_b.append(dict(
            y_in=ra[i]["y_out"], pooled_in=ra[i]["pooled_out"],
            bnsum=bnsum_tot, bnsq=bnsq_tot, ctxv=ctx_all[s],
            bng=bng_h, bnb=bnb_h, fb=fb_h, c2b=c2b_h,
            c1wT=c1wT_h, c1b=c1b_h, lng=lng_h, lnb=lnb_h, c2wT=c2wT_h,
            fwT=fwT_h, identb=identb, identf=identf,
        ))
    rb = _run(_CACHE["nc_b"], in_maps_b)

    out = np.zeros((B, C, H, W), np.float32)
    for i in range(8):
        s, hh = i // 2, i % 2
        oh = rb[i]["outh"].reshape(2, 128, OWN, W)
        out[s, 0:128, hh * OWN:(hh + 1) * OWN, :] = oh[0]
        out[s, 128:256, hh * OWN:(hh + 1) * OWN, :] = oh[1]
    return out



# revision 3
# speedup vs baseline: 1.3824x; 1.3824x over previous
"""Trainium2 Bass kernel for nn_BnDCN_Context (maxpool + DCNv2 + BN/ReLU + GCNet + 1x1 fusion).

Sharding: 8 cores = 4 samples x 2 row-halves; each core owns 32 pooled rows
(2048 output pixels) of one sample, with a 5-row halo band for the deformable
gather. Two launches; the host only sums ~6KB of per-core partial statistics
between them (BN batch stats + GCNet softmax partials = the collective step).

Phase A: maxpool -> offset/mod conv -> ONE dma_gather per (pixel-group, tap)
         from a doubled-row pixel-major DRAM map (map2[r] = [map r | map r+66])
         fetching all 4 bilinear corners -> per-corner diagonal-weighted
         transpose-matmuls on the PE array accumulate the combine directly in
         PSUM (channel-major) -> DCN matmul -> BN partial sums + GCNet
         attention partials.
Phase B: BN apply + ReLU, GCNet MLP + LayerNorm, 1x1 fusion with folded
         residual, output.
"""
import os
import numpy as np
import ml_dtypes

import concourse.bass as bass
import concourse.bacc as bacc
import concourse.tile as tile
from concourse import mybir
from concourse.bass_utils import run_bass_kernel_spmd

F32 = mybir.dt.float32
BF16 = mybir.dt.bfloat16
I16 = mybir.dt.int16
I32 = mybir.dt.int32
ALU = mybir.AluOpType
AF = mybir.ActivationFunctionType
BF = ml_dtypes.bfloat16

B, C, HI, WI = 4, 256, 128, 128
H = W = 64
HP = WP = 66
OWN = 32
NPIX = OWN * W                 # 2048
BAND = 42                      # local map rows (own 32 + 5 halo each side)
OWN0 = 5                       # local map row of first own data row
MPIX = BAND * HP               # 2772
MCH = (MPIX + 127) // 128      # 22 map chunks
MAP2_ROWS = 2816
QHI = float(BAND - 1)          # local row clip hi (41)
NTAP = 9
RR = C // 4                    # 64
N_TOT = float(B * H * W)       # 16384 (BN normalizer)
EPS = 1e-5

SIG = ((np.arange(128) % 16) * 8 + np.arange(128) // 16).astype(np.int64)


def build_phase_a():
    nc = bacc.Bacc("TRN2", target_bir_lowering=False)

    xin = nc.dram_tensor("xin", [2, 128, 84 * WI], BF16, kind="ExternalInput")
    p0xl8 = nc.dram_tensor("p0xl8", [128, 16 * NTAP], F32, kind="ExternalInput")
    p0yl8 = nc.dram_tensor("p0yl8", [128, 16 * NTAP], F32, kind="ExternalInput")
    p0xs = nc.dram_tensor("p0xs", [128, 16 * NTAP], F32, kind="ExternalInput")
    p0ys = nc.dram_tensor("p0ys", [128, 16 * NTAP], F32, kind="ExternalInput")
    ownm = nc.dram_tensor("ownm", [128, MCH], F32, kind="ExternalInput")
    cmb = nc.dram_tensor("cmb", [128, 1], F32, kind="ExternalInput")
    pmw = nc.dram_tensor("pmw", [2, 128, NTAP * 27], BF16, kind="ExternalInput")
    pmb = nc.dram_tensor("pmb", [1, 27], BF16, kind="ExternalInput")
    dcnw = nc.dram_tensor("dcnw", [2, 128, NTAP * C], BF16, kind="ExternalInput")
    dcnb = nc.dram_tensor("dcnb", [1, C], BF16, kind="ExternalInput")
    cmw = nc.dram_tensor("cmw", [2, 128], BF16, kind="ExternalInput")
    identb = nc.dram_tensor("identb", [128, 128], BF16, kind="ExternalInput")
    identf = nc.dram_tensor("identf", [128, 128], F32, kind="ExternalInput")

    y_out = nc.dram_tensor("y_out", [2, 128, NPIX], BF16, kind="ExternalOutput")
    pooled_out = nc.dram_tensor("pooled_out", [2, 128, NPIX], BF16, kind="ExternalOutput")
    stats = nc.dram_tensor("stats", [1, 1032], F32, kind="ExternalOutput")

    map2 = nc.dram_tensor("map2", [MAP2_ROWS, 512], BF16)
    wrapd = nc.dram_tensor("wrapd", [16, 2048], I16)

    with tile.TileContext(nc) as tc:
        with tc.tile_pool(name="singles", bufs=1) as singles, \
             tc.tile_pool(name="workp", bufs=int(os.environ.get("WB", "2"))) as workp, \
             tc.tile_pool(name="gpool", bufs=int(os.environ.get("GB", "2"))) as gpool, \
             tc.tile_pool(name="xop", bufs=int(os.environ.get("XB", "2"))) as xop, \
             tc.tile_pool(name="dpool", bufs=int(os.environ.get("DB", "3"))) as dpool:

            # ----- constants -----
            sb_p0xl8 = singles.tile([128, 16, NTAP], F32)
            sb_p0yl8 = singles.tile([128, 16, NTAP], F32)
            sb_p0xs = singles.tile([128, 16, NTAP], F32)
            sb_p0ys = singles.tile([128, 16, NTAP], F32)
            for t, d in ((sb_p0xl8, p0xl8), (sb_p0yl8, p0yl8), (sb_p0xs, p0xs), (sb_p0ys, p0ys)):
                nc.sync.dma_start(out=t, in_=d[:, :])
            sb_own = singles.tile([128, MCH], F32)
            nc.sync.dma_start(out=sb_own, in_=ownm[:, :])
            sb_cmb = singles.tile([128, 1], F32)
            nc.sync.dma_start(out=sb_cmb, in_=cmb[:, :])
            sb_pmw = singles.tile([128, 2, NTAP, 27], BF16)
            for ch in range(2):
                nc.sync.dma_start(out=sb_pmw[:, ch],
                                  in_=pmw[ch].rearrange("p (n o) -> p n o", n=NTAP))
            sb_pmb = singles.tile([1, 27], BF16)
            nc.sync.dma_start(out=sb_pmb, in_=pmb[:, :])
            sb_dcnw = singles.tile([128, 2, NTAP, C], BF16)
            for ch in range(2):
                nc.sync.dma_start(out=sb_dcnw[:, ch],
                                  in_=dcnw[ch].rearrange("p (n o) -> p n o", n=NTAP))
            sb_dcnb = singles.tile([1, C], BF16)
            nc.sync.dma_start(out=sb_dcnb, in_=dcnb[:, :])
            sb_cmw = singles.tile([128, 2], BF16)
            nc.sync.dma_start(out=sb_cmw, in_=cmw.rearrange("a p -> p a"))
            sb_idb = singles.tile([128, 128], BF16)
            nc.sync.dma_start(out=sb_idb, in_=identb[:, :])
            sb_idf = singles.tile([128, 128], F32)
            nc.sync.dma_start(out=sb_idf, in_=identf[:, :])
            sb_ones = singles.tile([1, 512], BF16)
            nc.vector.memset(sb_ones, 1.0)
            ones_col = singles.tile([128, 1], BF16)
            nc.vector.memset(ones_col, 1.0)

            # ----- pooling into padded band map (channel-major bf16) -----
            band = [singles.tile([128, BAND, HP], BF16, tag=f"band{c_}", name=f"band{c_}") for c_ in range(2)]
            with tc.tile_pool(name="xrawp", bufs=1) as xrawp:
                xraw = [xrawp.tile([128, 84 * WI], BF16, tag=f"xr{c_}", name=f"xr{c_}") for c_ in range(2)]
                for ch in range(2):
                    nc.sync.dma_start(out=xraw[ch], in_=xin[ch])
                    nc.vector.memset(band[ch], 0.0)
                    for rc in range(6):  # 7 pooled rows per chunk
                        rowmax = workp.tile([128, 7, WI], BF16, tag="rowmax")
                        even = bass.AP(tensor=xraw[ch].tensor, offset=xraw[ch].offset + rc * 14 * WI,
                                       ap=[xraw[ch].ap[0], [2 * WI, 7], [1, WI]])
                        odd = bass.AP(tensor=xraw[ch].tensor, offset=xraw[ch].offset + rc * 14 * WI + WI,
                                      ap=[xraw[ch].ap[0], [2 * WI, 7], [1, WI]])
                        nc.vector.tensor_tensor(out=rowmax, in0=even, in1=odd, op=ALU.max)
                        ceven = bass.AP(tensor=rowmax.tensor, offset=rowmax.offset,
                                        ap=[rowmax.ap[0], [WI, 7], [2, W]])
                        codd = bass.AP(tensor=rowmax.tensor, offset=rowmax.offset + 1,
                                       ap=[rowmax.ap[0], [WI, 7], [2, W]])
                        dst = bass.AP(tensor=band[ch].tensor,
                                      offset=band[ch].offset + (rc * 7) * HP + 1,
                                      ap=[band[ch].ap[0], [HP, 7], [1, W]])
                        nc.vector.tensor_tensor(out=dst, in0=ceven, in1=codd, op=ALU.max)

                # ----- pooled own rows -> DRAM (phase B) -----
                for ch in range(2):
                    src = bass.AP(tensor=band[ch].tensor,
                                  offset=band[ch].offset + OWN0 * HP + 1,
                                  ap=[band[ch].ap[0], [HP, OWN], [1, W]])
                    nc.sync.dma_start(out=pooled_out[ch], in_=src)

            bandf = [band[c_].rearrange("p a b -> p (a b)") for c_ in range(2)]

            # ----- pixel-major map (SBUF) + GCNet attention partials -----
            mapsb = singles.tile([128, MCH, 256], BF16)
            nc.vector.memset(mapsb, 0.0)
            off_sb = singles.tile([27, NPIX], F32)
            offnat = singles.tile([128, 16, 27], F32)
            offsig = singles.tile([128, 16, 27], F32)
            off_sg = singles.tile([27, NPIX], F32)
            S = singles.tile([128, 256], F32)
            idxw = singles.tile([128, 2048], I16)

            with tc.tile_pool(name="psMap", bufs=1, space="PSUM") as psMap, \
                 tc.tile_pool(name="psCTX", bufs=1, space="PSUM") as psCTX:
                ctx_ps = psCTX.tile([1, 257], F32)
                for m in range(MCH):
                    valid = 128 if m < MCH - 1 else MPIX - 128 * (MCH - 1)
                    tp = psMap.tile([128, 256], BF16, tag="tp", bufs=2)
                    for ch in range(2):
                        nc.tensor.transpose(tp[:valid, ch * 128:(ch + 1) * 128],
                                            bandf[ch][:, m * 128: m * 128 + valid], sb_idb)
                    nc.scalar.copy(mapsb[:valid, m, :], tp[:valid])
                    mk = psMap.tile([128, 1], F32, tag="mk")
                    for ch in range(2):
                        nc.tensor.matmul(mk[:valid], bandf[ch][:, m * 128: m * 128 + valid],
                                         sb_cmw[:, ch:ch + 1],
                                         start=(ch == 0), stop=(ch == 1))
                    e_f = workp.tile([128, 1], F32, tag="e_f")
                    nc.scalar.activation(out=e_f[:valid], in_=mk[:valid], func=AF.Exp,
                                         bias=sb_cmb[:valid], scale=1.0)
                    e_b = workp.tile([128, 1], BF16, tag="e_b")
                    nc.vector.tensor_tensor(out=e_b[:valid], in0=e_f[:valid],
                                            in1=sb_own[:valid, m:m + 1], op=ALU.mult)
                    nc.tensor.matmul(ctx_ps[:, 0:256], e_b[:valid], mapsb[:valid, m, :],
                                     start=(m == 0), stop=(m == MCH - 1))
                    nc.tensor.matmul(ctx_ps[:, 256:257], e_b[:valid], ones_col[:valid],
                                     start=(m == 0), stop=(m == MCH - 1))
                ctx_sb = workp.tile([1, 257], F32, tag="ctxsb")
                nc.vector.tensor_copy(ctx_sb, ctx_ps)
                nc.sync.dma_start(out=bass.AP(tensor=stats, offset=512, ap=[[1, 1], [1, 257]]),
                                  in_=ctx_sb)

                # ----- map2 doubled-row writes (3 DMAs) -----
                nc.sync.dma_start(
                    out=bass.AP(tensor=map2, offset=0, ap=[[512, 128], [512 * 128, MCH], [1, 256]]),
                    in_=mapsb[:, :, :])
                nc.sync.dma_start(
                    out=bass.AP(tensor=map2, offset=256, ap=[[512, 62], [512 * 128, MCH], [1, 256]]),
                    in_=mapsb[66:128, :, :])
                nc.sync.dma_start(
                    out=bass.AP(tensor=map2, offset=62 * 512 + 256,
                                ap=[[512, 66], [512 * 128, MCH - 1], [1, 256]]),
                    in_=mapsb[0:66, 1:MCH, :])

                # ----- offset/mod conv (27 ch) -----
                for pt in range(4):
                    ps = psMap.tile([27, 512], F32, tag="offps")
                    first = True
                    for ch in range(2):
                        for n in range(NTAP):
                            dy, dx = n // 3, n % 3
                            rhs = bass.AP(tensor=band[ch].tensor,
                                          offset=band[ch].offset + (OWN0 - 1 + 8 * pt + dy) * HP + dx,
                                          ap=[band[ch].ap[0], [HP, 8], [1, W]])
                            nc.tensor.matmul(ps, sb_pmw[:, ch, n], rhs, start=first, stop=False)
                            first = False
                    nc.tensor.matmul(ps, sb_pmb, sb_ones, start=False, stop=True)
                    nc.scalar.copy(off_sb[:, pt * 512:(pt + 1) * 512], ps)

                # ----- off transposes: natural + sigma layouts -----
                for t in range(16):
                    srcp = bass.AP(tensor=off_sb.tensor, offset=off_sb.offset + t * 128,
                                   ap=[off_sb.ap[0], [1, 8], [8, 16]])
                    nc.vector.tensor_copy(off_sg[:, t * 128:(t + 1) * 128], srcp)
                for t in range(16):
                    tpn = psMap.tile([128, 27], F32, tag="offT")
                    nc.tensor.transpose(tpn, off_sb[:, t * 128:(t + 1) * 128], sb_idf[0:27, 0:27])
                    nc.vector.tensor_copy(offnat[:, t], tpn)
                    tps = psMap.tile([128, 27], F32, tag="offT")
                    nc.tensor.transpose(tps, off_sg[:, t * 128:(t + 1) * 128], sb_idf[0:27, 0:27])
                    nc.vector.tensor_copy(offsig[:, t], tps)

                # ----- index math (natural layout): single pair (qlx) -----
                shp = [128, 16, NTAP]
                fxm8 = workp.tile(shp, F32, tag="im1")
                fym8 = workp.tile(shp, F32, tag="im2")
                ii = workp.tile(shp, I32, tag="imi")
                for (dst_, sl) in ((fxm8, 0), (fym8, NTAP)):
                    nc.vector.tensor_scalar_add(dst_, offnat[:, :, sl:sl + NTAP], 7.5)
                    nc.vector.tensor_copy(ii, dst_)
                    nc.vector.tensor_copy(dst_, ii)
                qlx = workp.tile(shp, F32, tag="im3")
                qly = workp.tile(shp, F32, tag="im4")
                nc.vector.tensor_tensor(out=qlx, in0=fxm8, in1=sb_p0xl8, op=ALU.add)
                nc.vector.tensor_scalar(out=qlx, in0=qlx, scalar1=0.0, scalar2=QHI,
                                        op0=ALU.max, op1=ALU.min)
                nc.vector.tensor_tensor(out=qly, in0=fym8, in1=sb_p0yl8, op=ALU.add)
                nc.vector.tensor_scalar(out=qly, in0=qly, scalar1=0.0, scalar2=65.0,
                                        op0=ALU.max, op1=ALU.min)
                # idx staging S [128, 256] f32, layout v = g*128 + n*8 + tl
                for g in range(2):
                    src0 = bass.AP(tensor=qlx.tensor, offset=qlx.offset + g * 72,
                                   ap=[qlx.ap[0], [9, 8], [1, NTAP]])
                    src1 = bass.AP(tensor=qly.tensor, offset=qly.offset + g * 72,
                                   ap=[qly.ap[0], [9, 8], [1, NTAP]])
                    dstS = bass.AP(tensor=S.tensor, offset=S.offset + g * 128,
                                   ap=[S.ap[0], [1, 8], [8, NTAP]])
                    nc.vector.scalar_tensor_tensor(out=dstS, in0=src0, scalar=66.0, in1=src1,
                                                   op0=ALU.mult, op1=ALU.add)
                # S -> T -> wrapped dram -> idxw (replicated via stride-0 read)
                for ck in range(2):
                    tps = psMap.tile([128, 128], F32, tag="ST")
                    nc.tensor.transpose(tps, S[:, ck * 128:(ck + 1) * 128], sb_idf)
                    ti = workp.tile([128, 128], I16, tag="Ti")
                    nc.vector.tensor_copy(ti, tps)
                    dst = bass.AP(tensor=wrapd, offset=ck * 1024,
                                  ap=[[8, 128], [2048, 16], [1, 8]])
                    src = bass.AP(tensor=ti.tensor, offset=ti.offset,
                                  ap=[ti.ap[0], [8, 16], [1, 8]])
                    nc.sync.dma_start(out=dst, in_=src)
                nc.sync.dma_start(out=idxw[:, :],
                                  in_=bass.AP(tensor=wrapd, offset=0, ap=[[0, 8], [1, 16 * 2048]]))

            # ----- weight math (sigma layout) -----
            shp = [128, 16, NTAP]
            fxs = workp.tile(shp, F32, tag="wm1")
            fys = workp.tile(shp, F32, tag="wm2")
            iis = workp.tile(shp, I32, tag="wmi")
            for (dst_, sl) in ((fxs, 0), (fys, NTAP)):
                nc.vector.tensor_scalar_add(dst_, offsig[:, :, sl:sl + NTAP], 7.5)
                nc.vector.tensor_copy(iis, dst_)
                nc.vector.tensor_copy(dst_, iis)
                nc.vector.tensor_scalar_add(dst_, dst_, -8.0)   # floor(off)
            pxc = workp.tile(shp, F32, tag="wm3")
            pyc = workp.tile(shp, F32, tag="wm4")
            nc.vector.tensor_tensor(out=pxc, in0=offsig[:, :, 0:NTAP], in1=sb_p0xs, op=ALU.add)
            nc.vector.tensor_scalar(out=pxc, in0=pxc, scalar1=0.0, scalar2=65.0,
                                    op0=ALU.max, op1=ALU.min)
            nc.vector.tensor_tensor(out=pyc, in0=offsig[:, :, NTAP:2 * NTAP], in1=sb_p0ys, op=ALU.add)
            nc.vector.tensor_scalar(out=pyc, in0=pyc, scalar1=0.0, scalar2=65.0,
                                    op0=ALU.max, op1=ALU.min)
            qlxg = workp.tile(shp, F32, tag="wm5")
            qlyg = workp.tile(shp, F32, tag="wm6")
            nc.vector.tensor_tensor(out=qlxg, in0=fxs, in1=sb_p0xs, op=ALU.add)
            nc.vector.tensor_scalar(out=qlxg, in0=qlxg, scalar1=0.0, scalar2=65.0,
                                    op0=ALU.max, op1=ALU.min)
            nc.vector.tensor_tensor(out=qlyg, in0=fys, in1=sb_p0ys, op=ALU.add)
            nc.vector.tensor_scalar(out=qlyg, in0=qlyg, scalar1=0.0, scalar2=65.0,
                                    op0=ALU.max, op1=ALU.min)
            qrxg = workp.tile(shp, F32, tag="wm7")
            qryg = workp.tile(shp, F32, tag="wm8")
            nc.vector.tensor_scalar(out=qrxg, in0=qlxg, scalar1=1.0, scalar2=65.0,
                                    op0=ALU.add, op1=ALU.min)
            nc.vector.tensor_scalar(out=qryg, in0=qlyg, scalar1=1.0, scalar2=65.0,
                                    op0=ALU.add, op1=ALU.min)
            wxl = workp.tile(shp, F32, tag="wm9")
            wyl = workp.tile(shp, F32, tag="wm10")
            wxr = workp.tile(shp, F32, tag="wm11")
            wyr = workp.tile(shp, F32, tag="wm12")
            nc.vector.scalar_tensor_tensor(out=wxl, in0=qlxg, scalar=1.0, in1=pxc,
                                           op0=ALU.add, op1=ALU.subtract)
            nc.vector.scalar_tensor_tensor(out=wyl, in0=qlyg, scalar=1.0, in1=pyc,
                                           op0=ALU.add, op1=ALU.subtract)
            nc.vector.scalar_tensor_tensor(out=wxr, in0=qrxg, scalar=-1.0, in1=pxc,
                                           op0=ALU.mult, op1=ALU.add)
            nc.vector.tensor_scalar_add(wxr, wxr, 1.0)
            nc.vector.scalar_tensor_tensor(out=wyr, in0=qryg, scalar=-1.0, in1=pyc,
                                           op0=ALU.mult, op1=ALU.add)
            nc.vector.tensor_scalar_add(wyr, wyr, 1.0)
            modv = workp.tile(shp, F32, tag="wm13")
            nc.scalar.activation(out=modv, in_=offsig[:, :, 2 * NTAP:3 * NTAP],
                                 func=AF.Sigmoid, bias=0.0, scale=1.0)
            nc.vector.tensor_tensor(out=wxl, in0=wxl, in1=modv, op=ALU.mult)
            nc.vector.tensor_tensor(out=wxr, in0=wxr, in1=modv, op=ALU.mult)
            wA = singles.tile(shp, F32)
            wB = singles.tile(shp, F32)
            wC = singles.tile(shp, F32)
            wD = singles.tile(shp, F32)
            nc.vector.tensor_tensor(out=wA, in0=wxl, in1=wyl, op=ALU.mult)
            nc.vector.tensor_tensor(out=wB, in0=wxl, in1=wyr, op=ALU.mult)
            nc.vector.tensor_tensor(out=wC, in0=wxr, in1=wyl, op=ALU.mult)
            nc.vector.tensor_tensor(out=wD, in0=wxr, in1=wyr, op=ALU.mult)

            # ----- gather / diag-weighted corner combine / DCN matmul -----
            y_sb = [singles.tile([128, NPIX], BF16, tag=f"ysb{c_}", name=f"ysb{c_}") for c_ in range(2)]
            s1parts = [singles.tile([128, 4], F32, tag=f"s1p{c_}", name=f"s1p{c_}") for c_ in range(2)]
            map_ap = bass.AP(tensor=map2, offset=0, ap=[[512, MAP2_ROWS - 1], [1, 1024]])
            wk = (wA, wC, wB, wD)  # chunk order: (ql,ql), (ql+1,ql), (ql,ql+1), (ql+1,ql+1)
            with tc.tile_pool(name="psXO", bufs=1, space="PSUM") as psXO, \
                 tc.tile_pool(name="psY", bufs=1, space="PSUM") as psY:
                for g in range(2):
                    yps = [psY.tile([128, 512], F32, tag=f"yps{h}{o}", name=f"yps{h}{o}")
                           for h in range(2) for o in range(2)]
                    for n in range(NTAP):
                        G = gpool.tile([128, 8, 1024], BF16, tag="G")
                        blk = g * 16 + n
                        nc.gpsimd.dma_gather(
                            out_ap=G[:, :, :], in_ap=map_ap,
                            idxs_ap=idxw[:, blk * 64:(blk + 1) * 64],
                            num_idxs=1024, num_idxs_reg=1024,
                            elem_size=1024, elem_step=512)
                        for h in range(2):
                            diags = dpool.tile([128, 16, 128], BF16, tag="diag")
                            for tl4 in range(4):
                                t_abs = g * 8 + h * 4 + tl4
                                for k in range(4):
                                    nc.vector.tensor_scalar_mul(diags[:, tl4 * 4 + k], sb_idb,
                                                                wk[k][:, t_abs, n:n + 1])
                            xoc = psXO.tile([128, 2, 512], F32, tag=f"xo{h}", name=f"xo{h}")
                            for tl4 in range(4):
                                for ch in range(2):
                                    for k in range(4):
                                        nc.tensor.matmul(
                                            xoc[:, ch, tl4 * 128:(tl4 + 1) * 128],
                                            G[:, h * 4 + tl4, k * 256 + ch * 128: k * 256 + ch * 128 + 128],
                                            diags[:, tl4 * 4 + k, :],
                                            start=(k == 0), stop=(k == 3))
                            xos = xop.tile([128, 2, 512], BF16, tag=f"xos{h}")
                            nc.scalar.copy(xos, xoc)
                            for ch in range(2):
                                for o in range(2):
                                    nc.tensor.matmul(yps[h * 2 + o],
                                                     sb_dcnw[:, ch, n, o * 128:(o + 1) * 128],
                                                     xos[:, ch],
                                                     start=(n == 0 and ch == 0), stop=False)
                    for h in range(2):
                        for o in range(2):
                            nc.tensor.matmul(yps[h * 2 + o], sb_dcnb[:, o * 128:(o + 1) * 128],
                                             sb_ones, start=False, stop=True)
                            # un-permute sigma on the copy out; BN sum rides accum_out
                            dsty = bass.AP(tensor=y_sb[o].tensor,
                                           offset=y_sb[o].offset + (g * 2 + h) * 512,
                                           ap=[y_sb[o].ap[0], [128, 4], [1, 8], [8, 16]])
                            srcy = bass.AP(tensor=yps[h * 2 + o].tensor,
                                           offset=yps[h * 2 + o].offset,
                                           ap=[yps[h * 2 + o].ap[0], [128, 4], [16, 8], [1, 16]])
                            nc.scalar.activation(out=dsty, in_=srcy, func=AF.Copy,
                                                 accum_out=s1parts[o][:, g * 2 + h: g * 2 + h + 1])

            # ----- BN partial sums + outputs -----
            scratch = workp.tile([128, NPIX], BF16, tag="scr")
            stat4 = workp.tile([128, 4], F32, tag="stat4")
            for o in range(2):
                nc.vector.tensor_reduce(stat4[:, o:o + 1], s1parts[o],
                                        axis=mybir.AxisListType.X, op=ALU.add)
                nc.scalar.activation(out=scratch, in_=y_sb[o], func=AF.Square,
                                     accum_out=stat4[:, 2 + o:3 + o])
                nc.sync.dma_start(out=y_out[o], in_=y_sb[o])
            nc.sync.dma_start(out=bass.AP(tensor=stats, offset=0, ap=[[1, 128], [128, 4]]),
                              in_=stat4)
    nc.compile()
    return nc


def build_phase_b():
    nc = bacc.Bacc("TRN2", target_bir_lowering=False)
    y_in = nc.dram_tensor("y_in", [2, 128, NPIX], BF16, kind="ExternalInput")
    pooled_in = nc.dram_tensor("pooled_in", [2, 128, NPIX], BF16, kind="ExternalInput")
    # packed per-channel params: [bnsum, bnsq, ctxv, bng, bnb, fb, c2b, pad]
    pk = nc.dram_tensor("pk", [2, 128, 8], F32, kind="ExternalInput")
    c1wT = nc.dram_tensor("c1wT", [2, 128, RR], BF16, kind="ExternalInput")
    c1b = nc.dram_tensor("c1b", [RR, 1], F32, kind="ExternalInput")
    lnpk = nc.dram_tensor("lnpk", [1, 2 * RR], F32, kind="ExternalInput")
    c2wT = nc.dram_tensor("c2wT", [RR, C], BF16, kind="ExternalInput")
    fwT = nc.dram_tensor("fwT", [128, 8, 128], BF16, kind="ExternalInput")
    identb = nc.dram_tensor("identb", [128, 128], BF16, kind="ExternalInput")
    identf = nc.dram_tensor("identf", [128, 128], F32, kind="ExternalInput")

    outh = nc.dram_tensor("outh", [2, 128, NPIX], F32, kind="ExternalOutput")

    with tile.TileContext(nc) as tc:
        with tc.tile_pool(name="singles", bufs=1) as singles, \
             tc.tile_pool(name="workp", bufs=2) as workp, \
             tc.tile_pool(name="ps", bufs=1, space="PSUM") as ps, \
             tc.tile_pool(name="psf", bufs=4, space="PSUM") as psf:
            ysb = singles.tile([128, 2, NPIX], BF16)
            psb = singles.tile([128, 2, NPIX], BF16)
            for t, d in ((ysb, y_in), (psb, pooled_in)):
                nc.sync.dma_start(
                    out=t, in_=bass.AP(tensor=d, offset=0,
                                       ap=[[NPIX, 128], [128 * NPIX, 2], [1, NPIX]]))
            pks = singles.tile([128, 2, 8], F32)
            nc.sync.dma_start(
                out=pks, in_=bass.AP(tensor=pk, offset=0,
                                     ap=[[8, 128], [128 * 8, 2], [1, 8]]))
            sb_c1w = singles.tile([128, 2, RR], BF16)
            for ch in range(2):
                nc.sync.dma_start(out=sb_c1w[:, ch], in_=c1wT[ch])
            sb_c1b = singles.tile([RR, 1], F32)
            nc.sync.dma_start(out=sb_c1b, in_=c1b[:, :])
            sb_ln = singles.tile([1, 2 * RR], F32)
            nc.sync.dma_start(out=sb_ln, in_=lnpk[:, :])
            sb_c2w = singles.tile([RR, C], BF16)
            nc.sync.dma_start(out=sb_c2w, in_=c2wT[:, :])
            sb_fw = singles.tile([128, 8, 128], BF16)
            nc.sync.dma_start(out=sb_fw, in_=fwT[:, :])
            sb_idb = singles.tile([128, 128], BF16)
            nc.sync.dma_start(out=sb_idb, in_=identb[:, :])
            sb_idf = singles.tile([128, 128], F32)
            nc.sync.dma_start(out=sb_idf, in_=identf[:, :])
            epsv = singles.tile([128, 1], F32)
            nc.vector.memset(epsv, EPS)

            # BN scale/shift
            ybn = [singles.tile([128, NPIX], BF16, tag=f"ybn{c_}", name=f"ybn{c_}") for c_ in range(2)]
            zb = [singles.tile([128, NPIX], BF16, tag=f"z{c_}", name=f"zb{c_}") for c_ in range(2)]
            biasF = [workp.tile([128, 1], F32, tag=f"bf{c_}", name=f"biasF{c_}") for c_ in range(2)]
            for ch in range(2):
                mu = workp.tile([128, 1], F32, tag="mu")
                nc.vector.tensor_scalar_mul(mu, pks[:, ch, 0:1], 1.0 / N_TOT)
                s2n = workp.tile([128, 1], F32, tag="s2n")
                nc.vector.tensor_scalar_mul(s2n, pks[:, ch, 1:2], 1.0 / N_TOT)
                negmu = workp.tile([128, 1], F32, tag="negmu")
                nc.vector.tensor_scalar_mul(negmu, mu, -1.0)
                var = workp.tile([128, 1], F32, tag="var")
                nc.vector.scalar_tensor_tensor(out=var, in0=mu, scalar=negmu, in1=s2n,
                                               op0=ALU.mult, op1=ALU.add)
                std = workp.tile([128, 1], F32, tag="std")
                nc.scalar.activation(out=std, in_=var, func=AF.Sqrt, bias=epsv, scale=1.0)
                rstd = workp.tile([128, 1], F32, tag="rstd")
                nc.vector.reciprocal(rstd, std)
                scale = workp.tile([128, 1], F32, tag="scale")
                nc.vector.tensor_tensor(out=scale, in0=pks[:, ch, 3:4], in1=rstd, op=ALU.mult)
                shift = workp.tile([128, 1], F32, tag="shift")
                nc.vector.scalar_tensor_tensor(out=shift, in0=scale, scalar=negmu,
                                               in1=pks[:, ch, 4:5], op0=ALU.mult, op1=ALU.add)
                nc.scalar.activation(out=ybn[ch], in_=ysb[:, ch], func=AF.Relu,
                                     bias=shift, scale=scale)

            # GCNet MLP
            ctxb = workp.tile([128, 2], BF16, tag="ctxb")
            for ch in range(2):
                nc.vector.tensor_copy(ctxb[:, ch:ch + 1], pks[:, ch, 2:3])
            t1p = ps.tile([RR, 1], F32)
            for ch in range(2):
                nc.tensor.matmul(t1p, sb_c1w[:, ch], ctxb[:, ch:ch + 1],
                                 start=(ch == 0), stop=(ch == 1))
            t1s = workp.tile([RR, 1], F32, tag="t1s")
            nc.vector.tensor_tensor(out=t1s, in0=t1p, in1=sb_c1b, op=ALU.add)
            t1tp = ps.tile([1, RR], F32)
            nc.tensor.transpose(t1tp, t1s, sb_idf[0:RR, 0:RR])
            t1t = workp.tile([1, RR], F32, tag="t1t")
            nc.vector.tensor_copy(t1t, t1tp)
            m1 = workp.tile([1, 1], F32, tag="m1")
            nc.vector.tensor_reduce(m1, t1t, axis=mybir.AxisListType.X, op=ALU.add)
            nc.vector.tensor_scalar_mul(m1, m1, -1.0 / RR)   # -mean
            cen = workp.tile([1, RR], F32, tag="cen")
            nc.vector.tensor_scalar_add(cen, t1t, m1)
            sq = workp.tile([1, RR], F32, tag="sq")
            v1 = workp.tile([1, 1], F32, tag="v1")
            nc.vector.scalar_tensor_tensor(out=sq, in0=cen, scalar=1.0, in1=cen,
                                           op0=ALU.mult, op1=ALU.mult, accum_out=v1)
            nc.vector.tensor_scalar_mul(v1, v1, 1.0 / RR)
            nc.scalar.activation(out=v1, in_=v1, func=AF.Sqrt, bias=epsv[0:1], scale=1.0)
            nc.vector.reciprocal(v1, v1)
            tn = workp.tile([1, RR], F32, tag="tn")
            nc.vector.tensor_scalar_mul(tn, cen, v1)
            nc.vector.tensor_tensor(out=tn, in0=tn, in1=sb_ln[:, 0:RR], op=ALU.mult)
            nc.vector.tensor_tensor(out=tn, in0=tn, in1=sb_ln[:, RR:2 * RR], op=ALU.add)
            tr = workp.tile([1, RR], BF16, tag="tr")
            nc.scalar.activation(out=tr, in_=tn, func=AF.Relu, bias=0.0, scale=1.0)
            trtp = ps.tile([RR, 1], BF16)
            nc.tensor.transpose(trtp, tr, sb_idb[0:1, 0:1])
            trt = workp.tile([RR, 1], BF16, tag="trt")
            nc.vector.tensor_copy(trt, trtp)
            for ch in range(2):
                tp2 = ps.tile([128, 1], F32, tag="tp2")
                nc.tensor.matmul(tp2, sb_c2w[:, ch * 128:(ch + 1) * 128], trt,
                                 start=True, stop=True)
                tv = workp.tile([128, 1], F32, tag="tv")
                nc.vector.tensor_tensor(out=tv, in0=tp2, in1=pks[:, ch, 6:7], op=ALU.add)
                nc.vector.tensor_tensor(out=biasF[ch], in0=pks[:, ch, 5:6], in1=tv, op=ALU.subtract)
                nc.vector.tensor_scalar_add(zb[ch], psb[:, ch], tv)

            # fusion
            outsb = [singles.tile([128, NPIX], F32, tag=f"o{c_}", name=f"outsb{c_}") for c_ in range(2)]
            rhs = [ybn[0], ybn[1], zb[0], zb[1]]
            for o in range(2):
                for pt in range(4):
                    pf = psf.tile([128, 512], F32, tag="pf")
                    for k in range(4):
                        nc.tensor.matmul(pf, sb_fw[:, k * 2 + o],
                                         rhs[k][:, pt * 512:(pt + 1) * 512],
                                         start=(k == 0), stop=(k == 3))
                    nc.scalar.activation(out=outsb[o][:, pt * 512:(pt + 1) * 512], in_=pf,
                                         func=AF.Identity, bias=biasF[o], scale=1.0)
                nc.sync.dma_start(out=outh[o], in_=outsb[o])
    nc.compile()
    return nc


# ---------------- host side ----------------
_CACHE = {}
EXEC_NS = []


def _run(nc, in_maps):
    if os.environ.get("KERNEL_SIM"):
        from concourse.bass_interp import CoreSim
        outs = []
        for i, im in enumerate(in_maps):
            sim = CoreSim(nc, require_finite=False, require_nnan=False)
            for k, v in im.items():
                sim.tensor(k)[:] = v
            sim.simulate(check_with_hw=False)
            out_allocs = {a.memorylocations[0].name: list(a.tensor_shape)
                          for a in nc.m.functions[0].allocations
                          if getattr(a, "kind", None) == "ExternalOutput"}
            outs.append({k: np.array(sim.mem_tensor(k)).reshape(shp)
                         for k, shp in out_allocs.items()})
            print(f"  sim core {i} done")
        return outs
    res = run_bass_kernel_spmd(nc, in_maps, core_ids=list(range(8)))
    if res.exec_time_ns is not None:
        EXEC_NS.append(res.exec_time_ns)
    return res.results


def _consts():
    if "c" in _CACHE:
        return _CACHE["c"]
    rng3 = np.arange(-1, 2)
    pnx = np.repeat(rng3, 3).astype(np.float32)   # tap n = (dy+1)*3+(dx+1)
    pny = np.tile(rng3, 3).astype(np.float32)
    p = np.arange(128)
    t = np.arange(16)
    s_nat = t[None, :] * 128 + p[:, None]          # [128,16]
    s_sig = t[None, :] * 128 + SIG[p][:, None]
    consts = {}
    for hh in range(2):
        g0 = 1 + 32 * hh
        r_nat = s_nat // 64
        c_nat = s_nat % 64
        r_sig = s_sig // 64
        c_sig = s_sig % 64
        consts[hh] = dict(
            p0xl8=(OWN0 + r_nat[:, :, None] + pnx[None, None, :] - 8.0).astype(np.float32).reshape(128, -1),
            p0yl8=(c_nat[:, :, None] + 1 + pny[None, None, :] - 8.0).astype(np.float32).reshape(128, -1),
            p0xs=(g0 + r_sig[:, :, None] + pnx[None, None, :]).astype(np.float32).reshape(128, -1),
            p0ys=(c_sig[:, :, None] + 1 + pny[None, None, :]).astype(np.float32).reshape(128, -1),
        )
    mp = np.arange(MCH * 128)
    mrow, mcol = mp // HP, mp % HP
    own = ((mrow >= OWN0) & (mrow < OWN0 + OWN) & (mcol >= 1) & (mcol < 65) & (mp < MPIX))
    ownm = own.astype(np.float32).reshape(MCH, 128).T.copy()   # [128, MCH]
    identb = np.eye(128, dtype=BF)
    identf = np.eye(128, dtype=np.float32)
    _CACHE["c"] = (consts, ownm, identb, identf)
    return _CACHE["c"]


def kernel(x, p_w, p_b, m_w, m_b, dcn_w, dcn_b, bn_g, bn_b,
           cm_w, cm_b, c1_w, c1_b, ln_g, ln_b, c2_w, c2_b, f_w, f_b):
    x = np.asarray(x, np.float32)
    consts, ownm, identb, identf = _consts()

    # weights prep
    pm = np.concatenate([np.asarray(p_w), np.asarray(m_w)], 0).astype(np.float32)  # [27,256,3,3]
    pmw = np.zeros((2, 128, NTAP * 27), BF)
    for ch in range(2):
        for n in range(NTAP):
            pmw[ch, :, n * 27:(n + 1) * 27] = pm[:, ch * 128:(ch + 1) * 128, n // 3, n % 3].T.astype(BF)
    pmb_h = np.concatenate([np.asarray(p_b), np.asarray(m_b)]).astype(BF)[None, :]
    dw = np.asarray(dcn_w, np.float32).reshape(C, C, NTAP)
    dcnw_h = np.zeros((2, 128, NTAP * C), BF)
    for ch in range(2):
        for n in range(NTAP):
            dcnw_h[ch, :, n * C:(n + 1) * C] = dw[:, ch * 128:(ch + 1) * 128, n].T.astype(BF)
    dcnb_h = np.asarray(dcn_b, np.float32).astype(BF)[None, :]
    cmw_h = np.asarray(cm_w, np.float32).reshape(C).astype(BF).reshape(2, 128)
    cmb_h = np.full((128, 1), float(np.asarray(cm_b).reshape(-1)[0]), np.float32)
    fw2 = np.asarray(f_w, np.float32).reshape(C, 2 * C)
    fwp = fw2.copy()
    fwp[:, C:] += np.eye(C, dtype=np.float32)
    fwT_h = np.zeros((128, 8, 128), BF)
    for kc in range(4):
        for oc in range(2):
            fwT_h[:, kc * 2 + oc, :] = fwp[oc * 128:(oc + 1) * 128, kc * 128:(kc + 1) * 128].T.astype(BF)
    c1w2 = np.asarray(c1_w, np.float32).reshape(RR, C)
    c1wT_h = np.stack([c1w2[:, ch * 128:(ch + 1) * 128].T.astype(BF) for ch in range(2)])
    c2w2 = np.asarray(c2_w, np.float32).reshape(C, RR)
    c2wT_h = c2w2.T.astype(BF)                      # [RR, C]
    c1b_h = np.asarray(c1_b, np.float32).reshape(RR, 1)
    lnpk_h = np.concatenate([np.asarray(ln_g, np.float32).reshape(RR),
                             np.asarray(ln_b, np.float32).reshape(RR)]).reshape(1, 2 * RR)

    xbf = x.astype(BF)
    in_maps_a = []
    for i in range(8):
        s, hh = i // 2, i % 2
        g0 = 1 + 32 * hh
        xin = np.zeros((2, 128, 84, WI), BF)
        for l in range(BAND):
            pr = g0 - 6 + l
            if 0 <= pr < 64:
                xin[:, :, 2 * l:2 * l + 2, :] = xbf[s].reshape(2, 128, HI, WI)[:, :, 2 * pr:2 * pr + 2, :]
        cc = consts[hh]
        in_maps_a.append(dict(
            xin=xin.reshape(2, 128, 84 * WI),
            p0xl8=cc["p0xl8"], p0yl8=cc["p0yl8"], p0xs=cc["p0xs"], p0ys=cc["p0ys"],
            ownm=ownm, cmb=cmb_h, pmw=pmw, pmb=pmb_h, dcnw=dcnw_h, dcnb=dcnb_h,
            cmw=cmw_h, identb=identb, identf=identf,
        ))

    if "nc_a" not in _CACHE:
        _CACHE["nc_a"] = build_phase_a()
        _CACHE["nc_b"] = build_phase_b()
    ra = _run(_CACHE["nc_a"], in_maps_a)

    st = np.stack([ra[i]["stats"][0] for i in range(8)])   # [8, 1032]
    bnsum_tot = st[:, 0:256].sum(0).reshape(2, 128).astype(np.float32)
    bnsq_tot = st[:, 256:512].sum(0).reshape(2, 128).astype(np.float32)
    ctx_all = []
    for s in range(4):
        p1 = st[2 * s, 512:768] + st[2 * s + 1, 512:768]
        z = st[2 * s, 768] + st[2 * s + 1, 768]
        ctx_all.append((p1 / z).reshape(2, 128).astype(np.float32))

    two = lambda v: np.asarray(v, np.float32).reshape(2, 128)
    bng_h, bnb_h, fb_h, c2b_h = two(bn_g), two(bn_b), two(f_b), two(c2_b)

    in_maps_b = []
    for i in range(8):
        s = i // 2
        pk = np.stack([bnsum_tot, bnsq_tot, ctx_all[s], bng_h, bnb_h, fb_h, c2b_h,
                       np.zeros((2, 128), np.float32)], axis=2)   # [2, 128, 8]
        in_maps_b.append(dict(
            y_in=ra[i]["y_out"], pooled_in=ra[i]["pooled_out"],
            pk=pk.astype(np.float32),
            c1wT=c1wT_h, c1b=c1b_h, lnpk=lnpk_h, c2wT=c2wT_h,
            fwT=fwT_h, identb=identb, identf=identf,
        ))
    rb = _run(_CACHE["nc_b"], in_maps_b)

    out = np.zeros((B, C, H, W), np.float32)
    for i in range(8):
        s, hh = i // 2, i % 2
        oh = rb[i]["outh"].reshape(2, 128, OWN, W)
        out[s, 0:128, hh * OWN:(hh + 1) * OWN, :] = oh[0]
        out[s, 128:256, hh * OWN:(hh + 1) * OWN, :] = oh[1]
    return out


# revision 22
# speedup vs baseline: 1.4385x; 1.0405x over previous
"""Trainium2 Bass kernel for nn_BnDCN_Context (maxpool + DCNv2 + BN/ReLU + GCNet + 1x1 fusion).

Sharding: 8 cores = 4 samples x 2 row-halves; each core owns 32 pooled rows
(2048 output pixels) of one sample, with a 5-row halo band for the deformable
gather. Two launches; the host only sums ~6KB of per-core partial statistics
between them (BN batch stats + GCNet softmax partials = the collective step).

Phase A: maxpool -> offset/mod conv -> ONE dma_gather per (pixel-group, tap)
         from a doubled-row pixel-major DRAM map (map2[r] = [map r | map r+66])
         fetching all 4 bilinear corners -> per-corner diagonal-weighted
         transpose-matmuls on the PE array accumulate the combine directly in
         PSUM (channel-major) -> DCN matmul -> BN partial sums + GCNet
         attention partials.
Phase B: BN apply + ReLU, GCNet MLP + LayerNorm, 1x1 fusion with folded
         residual, output.
"""
import os
import numpy as np
import ml_dtypes

import concourse.bass as bass
import concourse.bacc as bacc
import concourse.tile as tile
from concourse import mybir
from concourse.bass_utils import run_bass_kernel_spmd

F32 = mybir.dt.float32
BF16 = mybir.dt.bfloat16
FP8 = mybir.dt.float8e4
I16 = mybir.dt.int16
I32 = mybir.dt.int32
ALU = mybir.AluOpType
AF = mybir.ActivationFunctionType
BF = ml_dtypes.bfloat16
E4 = ml_dtypes.float8_e4m3
PMW_SCALE = 64.0   # offset/mod conv weights pre-scale (fp8 subnormal dodge)
DCN_SCALE = 32.0   # dcn weights pre-scale
DR = mybir.MatmulPerfMode.DoubleRow
FP8_MAP = os.environ.get("FP8_MAP", "0") == "1"   # gather map dtype
FP8_DCN = os.environ.get("FP8_DCN", "0") == "1"   # dcn weights + xo dtype (DoubleRow)
FP8_OFF = os.environ.get("FP8_OFF", "0") == "1"   # offset-conv weights + band (DoubleRow)
MAPDT = FP8 if FP8_MAP else BF16
MAPNP = E4 if FP8_MAP else BF

B, C, HI, WI = 4, 256, 128, 128
H = W = 64
HP = WP = 66
OWN = 32
NPIX = OWN * W                 # 2048
BAND = 42                      # local map rows (own 32 + 5 halo each side)
OWN0 = 5                       # local map row of first own data row
MPIX = BAND * HP               # 2772
MCH = (MPIX + 127) // 128      # 22 map chunks
MAP2_ROWS = 2816
QHI = float(BAND - 1)          # local row clip hi (41)
NTAP = 9
RR = C // 4                    # 64
N_TOT = float(B * H * W)       # 16384 (BN normalizer)
EPS = 1e-5

SIG = ((np.arange(128) % 16) * 8 + np.arange(128) // 16).astype(np.int64)


def build_phase_a():
    nc = bacc.Bacc("TRN2", target_bir_lowering=False)

    xin = nc.dram_tensor("xin", [2, 128, 84 * WI], BF16, kind="ExternalInput")
    p0xl8 = nc.dram_tensor("p0xl8", [128, 16 * NTAP], F32, kind="ExternalInput")
    p0yl8 = nc.dram_tensor("p0yl8", [128, 16 * NTAP], F32, kind="ExternalInput")
    p0xs = nc.dram_tensor("p0xs", [128, 16 * NTAP], F32, kind="ExternalInput")
    p0ys = nc.dram_tensor("p0ys", [128, 16 * NTAP], F32, kind="ExternalInput")
    ownm = nc.dram_tensor("ownm", [128, MCH], F32, kind="ExternalInput")
    cmb = nc.dram_tensor("cmb", [128, 1], F32, kind="ExternalInput")
    pmw = nc.dram_tensor("pmw", [2, 128, NTAP * 32], FP8 if FP8_OFF else BF16, kind="ExternalInput")
    pmb = nc.dram_tensor("pmb", [1, 27], BF16, kind="ExternalInput")
    dcnw = nc.dram_tensor("dcnw", [2, 128, NTAP * C], FP8 if FP8_DCN else BF16, kind="ExternalInput")
    dcnb = nc.dram_tensor("dcnb", [1, C], BF16, kind="ExternalInput")
    cmw = nc.dram_tensor("cmw", [2, 128], BF16, kind="ExternalInput")
    identb = nc.dram_tensor("identb", [128, 128], BF16, kind="ExternalInput")
    identf = nc.dram_tensor("identf", [128, 128], F32, kind="ExternalInput")

    y_out = nc.dram_tensor("y_out", [2, 128, NPIX], BF16, kind="ExternalOutput")
    pooled_out = nc.dram_tensor("pooled_out", [2, 128, NPIX], BF16, kind="ExternalOutput")
    stats = nc.dram_tensor("stats", [1, 1032], F32, kind="ExternalOutput")

    map2 = nc.dram_tensor("map2", [MAP2_ROWS, 512], MAPDT)
    wrapd = nc.dram_tensor("wrapd", [16, 2048], I16)

    with tile.TileContext(nc) as tc:
        with tc.tile_pool(name="singles", bufs=1) as singles, \
             tc.tile_pool(name="workp", bufs=int(os.environ.get("WB", "2"))) as workp, \
             tc.tile_pool(name="gpool", bufs=int(os.environ.get("GB", "3"))) as gpool, \
             tc.tile_pool(name="xop", bufs=int(os.environ.get("XB", "2"))) as xop, \
             tc.tile_pool(name="dpool", bufs=int(os.environ.get("DB", "3"))) as dpool:

            # ----- pooling into padded band map (channel-major bf16) -----
            band = [singles.tile([128, BAND, HP], BF16, tag=f"band{c_}", name=f"band{c_}") for c_ in range(2)]
            with tc.tile_pool(name="xrawp", bufs=2) as xrawp:
                for ch in range(2):
                    nc.vector.memset(bass.AP(tensor=band[ch].tensor, offset=band[ch].offset,
                                             ap=[band[ch].ap[0], [HP, BAND], [65, 2]]), 0.0)
                for ch in range(2):
                    for hf in range(2):
                        xr = xrawp.tile([128, 42 * WI], BF16, tag="xr")
                        nc.sync.dma_start(out=xr, in_=xin[ch][:, hf * 42 * WI:(hf + 1) * 42 * WI])
                        for r3 in range(3):  # 7 pooled rows per chunk
                            rc = hf * 3 + r3
                            rowmax = workp.tile([128, 7, WI], BF16, tag="rowmax")
                            even = bass.AP(tensor=xr.tensor, offset=xr.offset + r3 * 14 * WI,
                                           ap=[xr.ap[0], [2 * WI, 7], [1, WI]])
                            odd = bass.AP(tensor=xr.tensor, offset=xr.offset + r3 * 14 * WI + WI,
                                          ap=[xr.ap[0], [2 * WI, 7], [1, WI]])
                            nc.vector.tensor_tensor(out=rowmax, in0=even, in1=odd, op=ALU.max)
                            ceven = bass.AP(tensor=rowmax.tensor, offset=rowmax.offset,
                                            ap=[rowmax.ap[0], [WI, 7], [2, W]])
                            codd = bass.AP(tensor=rowmax.tensor, offset=rowmax.offset + 1,
                                           ap=[rowmax.ap[0], [WI, 7], [2, W]])
                            dst = bass.AP(tensor=band[ch].tensor,
                                          offset=band[ch].offset + (rc * 7) * HP + 1,
                                          ap=[band[ch].ap[0], [HP, 7], [1, W]])
                            nc.vector.tensor_tensor(out=dst, in0=ceven, in1=codd, op=ALU.max)

                # ----- pooled own rows -> DRAM (phase B) -----
                for ch in range(2):
                    src = bass.AP(tensor=band[ch].tensor,
                                  offset=band[ch].offset + OWN0 * HP + 1,
                                  ap=[band[ch].ap[0], [HP, OWN], [1, W]])
                    nc.sync.dma_start(out=pooled_out[ch], in_=src)

            # ----- constants -----
            sb_p0xl8 = singles.tile([128, 16, NTAP], F32)
            sb_p0yl8 = singles.tile([128, 16, NTAP], F32)
            sb_p0xs = singles.tile([128, 16, NTAP], F32)
            sb_p0ys = singles.tile([128, 16, NTAP], F32)
            for t, d in ((sb_p0xl8, p0xl8), (sb_p0yl8, p0yl8), (sb_p0xs, p0xs), (sb_p0ys, p0ys)):
                nc.sync.dma_start(out=t, in_=d[:, :])
            sb_own = singles.tile([128, MCH], F32)
            nc.sync.dma_start(out=sb_own, in_=ownm[:, :])
            sb_cmb = singles.tile([128, 1], F32)
            nc.sync.dma_start(out=sb_cmb, in_=cmb[:, :])
            sb_pmw = singles.tile([128, 2, NTAP, 32], FP8 if FP8_OFF else BF16)
            for ch in range(2):
                nc.sync.dma_start(out=sb_pmw[:, ch],
                                  in_=pmw[ch].rearrange("p (n o) -> p n o", n=NTAP))
            sb_pmb = singles.tile([1, 27], BF16)
            nc.sync.dma_start(out=sb_pmb, in_=pmb[:, :])
            sb_dcnw = singles.tile([128, 2, NTAP, C], FP8 if FP8_DCN else BF16)
            for ch in range(2):
                nc.sync.dma_start(out=sb_dcnw[:, ch],
                                  in_=dcnw[ch].rearrange("p (n o) -> p n o", n=NTAP))
            sb_dcnb = singles.tile([1, C], BF16)
            nc.sync.dma_start(out=sb_dcnb, in_=dcnb[:, :])
            sb_cmw = singles.tile([128, 2], BF16)
            nc.sync.dma_start(out=sb_cmw, in_=cmw.rearrange("a p -> p a"))
            sb_idb = singles.tile([128, 128], BF16)
            nc.sync.dma_start(out=sb_idb, in_=identb[:, :])
            sb_idf = singles.tile([128, 128], F32)
            nc.sync.dma_start(out=sb_idf, in_=identf[:, :])
            sb_ones = singles.tile([1, 544], BF16)
            nc.vector.memset(sb_ones, 1.0)
            ones_col = singles.tile([128, 1], BF16)
            nc.vector.memset(ones_col, 1.0)

            bandf = [band[c_].rearrange("p a b -> p (a b)") for c_ in range(2)]
            if FP8_OFF:
                BP = 2784  # BAND*HP (2772) padded to a 16B multiple
                bandf8 = singles.tile([128, 2, BP], FP8)
                bandf8s = singles.tile([128, 2, BP], FP8)
                for ch in range(2):
                    nc.scalar.copy(bandf8[:, ch, 0:MPIX], bandf[ch])
                    nc.scalar.copy(bandf8s[:, ch, 0:MPIX - 1], bandf[ch][:, 1:MPIX])

            # ----- pixel-major map (SBUF) + GCNet attention partials -----
            mapsb = singles.tile([128, MCH, 256], MAPDT)
            nc.vector.memset(mapsb[:, MCH - 1, :], 0.0)
            off_sb = singles.tile([27, NPIX], F32)
            offnat = singles.tile([128, 16, 27], F32)
            offsig = singles.tile([128, 16, 27], F32)
            off_sg = singles.tile([27, NPIX], F32)
            S = singles.tile([128, 256], F32)
            idxw = singles.tile([128, 2048], I16)

            # ----- offset/mod conv (27 ch) -----
            with tc.tile_pool(name="psMapB", bufs=1, space="PSUM") as psMapB:
                if FP8_OFF:
                    for pt in range(8):
                        ps = psMapB.tile([27, 264], F32, tag="offps", bufs=2)
                        for n in range(NTAP):
                            dy, dx = n // 3, n % 3
                            base = (OWN0 - 1 + 4 * pt + dy) * HP + dx
                            t8 = bandf8 if dx % 2 == 0 else bandf8s
                            if dx % 2 == 1:
                                base -= 1
                            rhs = bass.AP(tensor=t8.tensor, offset=t8.offset + base,
                                          ap=[t8.ap[0], [BP, 2], [1, 264]])
                            nc.tensor.matmul(ps, sb_pmw[:, :, n, 0:27], rhs, start=(n == 0),
                                             stop=False, perf_mode=DR)
                        nc.tensor.matmul(ps, sb_pmb, sb_ones[:, 0:264], start=False, stop=True)
                        dst_off = bass.AP(tensor=off_sb.tensor, offset=off_sb.offset + pt * 256,
                                          ap=[off_sb.ap[0], [64, 4], [1, 64]])
                        src_ps = bass.AP(tensor=ps.tensor, offset=ps.offset,
                                         ap=[ps.ap[0], [HP, 4], [1, 64]])
                        nc.scalar.activation(out=dst_off, in_=src_ps,
                                             func=AF.Copy, scale=1.0 / PMW_SCALE)
                else:
                    for pt in range(4):
                        ps = psMapB.tile([27, 512], F32, tag="offps", bufs=2)
                        first = True
                        for ch in range(2):
                            for n in range(NTAP):
                                dy, dx = n // 3, n % 3
                                rhs = bass.AP(tensor=band[ch].tensor,
                                              offset=band[ch].offset + (OWN0 - 1 + 8 * pt + dy) * HP + dx,
                                              ap=[band[ch].ap[0], [HP, 8], [1, W]])
                                nc.tensor.matmul(ps, sb_pmw[:, ch, n, 0:27], rhs, start=first, stop=False)
                                first = False
                        nc.tensor.matmul(ps, sb_pmb, sb_ones[:, 0:512], start=False, stop=True)
                        nc.scalar.activation(out=off_sb[:, pt * 512:(pt + 1) * 512], in_=ps,
                                             func=AF.Copy, scale=1.0 / PMW_SCALE)

                # ----- off transposes: natural + sigma layouts -----
                for t in range(16):
                    srcp = bass.AP(tensor=off_sb.tensor, offset=off_sb.offset + t * 128,
                                   ap=[off_sb.ap[0], [1, 8], [8, 16]])
                    nc.vector.tensor_copy(off_sg[:, t * 128:(t + 1) * 128], srcp)
                tpn = psMapB.tile([128, 16, 27], F32, tag="offTn")
                tpsg = psMapB.tile([128, 16, 27], F32, tag="offTs")
                for t in range(16):
                    nc.tensor.transpose(tpn[:, t], off_sb[:, t * 128:(t + 1) * 128], sb_idf[0:27, 0:27])
                    nc.tensor.transpose(tpsg[:, t], off_sg[:, t * 128:(t + 1) * 128], sb_idf[0:27, 0:27])
                nc.vector.tensor_copy(offnat, tpn)
                nc.vector.tensor_copy(offsig, tpsg)

                # ----- index math (natural layout): single pair (qlx) -----
                shp = [128, 16, NTAP]
                fxm8 = workp.tile(shp, F32, tag="im1")
                fym8 = workp.tile(shp, F32, tag="im2")
                ii = workp.tile(shp, I32, tag="imi")
                for (dst_, sl) in ((fxm8, 0), (fym8, NTAP)):
                    nc.vector.tensor_scalar_add(dst_, offnat[:, :, sl:sl + NTAP], 7.5)
                    nc.vector.tensor_copy(ii, dst_)
                    nc.vector.tensor_copy(dst_, ii)
                qlx = workp.tile(shp, F32, tag="im3")
                qly = workp.tile(shp, F32, tag="im4")
                nc.vector.tensor_tensor(out=qlx, in0=fxm8, in1=sb_p0xl8, op=ALU.add)
                nc.vector.tensor_scalar(out=qlx, in0=qlx, scalar1=0.0, scalar2=QHI,
                                        op0=ALU.max, op1=ALU.min)
                nc.vector.tensor_tensor(out=qly, in0=fym8, in1=sb_p0yl8, op=ALU.add)
                nc.vector.tensor_scalar(out=qly, in0=qly, scalar1=0.0, scalar2=65.0,
                                        op0=ALU.max, op1=ALU.min)
                # idx staging S [128, 256] f32, layout v = g*128 + n*8 + tl
                for g in range(2):
                    src0 = bass.AP(tensor=qlx.tensor, offset=qlx.offset + g * 72,
                                   ap=[qlx.ap[0], [9, 8], [1, NTAP]])
                    src1 = bass.AP(tensor=qly.tensor, offset=qly.offset + g * 72,
                                   ap=[qly.ap[0], [9, 8], [1, NTAP]])
                    dstS = bass.AP(tensor=S.tensor, offset=S.offset + g * 128,
                                   ap=[S.ap[0], [1, 8], [8, NTAP]])
                    nc.vector.scalar_tensor_tensor(out=dstS, in0=src0, scalar=66.0, in1=src1,
                                                   op0=ALU.mult, op1=ALU.add)
                # S -> T -> wrapped dram -> idxw (replicated via stride-0 read)
                for ck in range(2):
                    tps = psMapB.tile([128, 128], F32, tag="ST")
                    nc.tensor.transpose(tps, S[:, ck * 128:(ck + 1) * 128], sb_idf)
                    ti = workp.tile([128, 128], I16, tag="Ti")
                    nc.vector.tensor_copy(ti, tps)
                    dst = bass.AP(tensor=wrapd, offset=ck * 1024,
                                  ap=[[8, 128], [2048, 16], [1, 8]])
                    src = bass.AP(tensor=ti.tensor, offset=ti.offset,
                                  ap=[ti.ap[0], [8, 16], [1, 8]])
                    nc.sync.dma_start(out=dst, in_=src)
                nc.sync.dma_start(out=idxw[:, :],
                                  in_=bass.AP(tensor=wrapd, offset=0, ap=[[0, 8], [1, 16 * 2048]]))

            # ----- weight math (sigma layout) -----
            shp = [128, 16, NTAP]
            fxs = workp.tile(shp, F32, tag="wm1")
            fys = workp.tile(shp, F32, tag="wm2")
            iis = workp.tile(shp, I32, tag="wmi")
            for (dst_, sl) in ((fxs, 0), (fys, NTAP)):
                nc.vector.tensor_scalar_add(dst_, offsig[:, :, sl:sl + NTAP], 7.5)
                nc.vector.tensor_copy(iis, dst_)
                nc.vector.tensor_copy(dst_, iis)
                nc.vector.tensor_scalar_add(dst_, dst_, -8.0)   # floor(off)
            pxc = workp.tile(shp, F32, tag="wm3")
            pyc = workp.tile(shp, F32, tag="wm4")
            nc.vector.tensor_tensor(out=pxc, in0=offsig[:, :, 0:NTAP], in1=sb_p0xs, op=ALU.add)
            nc.vector.tensor_scalar(out=pxc, in0=pxc, scalar1=0.0, scalar2=65.0,
                                    op0=ALU.max, op1=ALU.min)
            nc.vector.tensor_tensor(out=pyc, in0=offsig[:, :, NTAP:2 * NTAP], in1=sb_p0ys, op=ALU.add)
            nc.vector.tensor_scalar(out=pyc, in0=pyc, scalar1=0.0, scalar2=65.0,
                                    op0=ALU.max, op1=ALU.min)
            qlxg = workp.tile(shp, F32, tag="wm5")
            qlyg = workp.tile(shp, F32, tag="wm6")
            nc.vector.tensor_tensor(out=qlxg, in0=fxs, in1=sb_p0xs, op=ALU.add)
            nc.vector.tensor_scalar(out=qlxg, in0=qlxg, scalar1=0.0, scalar2=65.0,
                                    op0=ALU.max, op1=ALU.min)
            nc.vector.tensor_tensor(out=qlyg, in0=fys, in1=sb_p0ys, op=ALU.add)
            nc.vector.tensor_scalar(out=qlyg, in0=qlyg, scalar1=0.0, scalar2=65.0,
                                    op0=ALU.max, op1=ALU.min)
            qrxg = workp.tile(shp, F32, tag="wm7")
            qryg = workp.tile(shp, F32, tag="wm8")
            nc.vector.tensor_scalar(out=qrxg, in0=qlxg, scalar1=1.0, scalar2=65.0,
                                    op0=ALU.add, op1=ALU.min)
            nc.vector.tensor_scalar(out=qryg, in0=qlyg, scalar1=1.0, scalar2=65.0,
                                    op0=ALU.add, op1=ALU.min)
            wxl = workp.tile(shp, F32, tag="wm9")
            wyl = workp.tile(shp, F32, tag="wm10")
            wxr = workp.tile(shp, F32, tag="wm11")
            wyr = workp.tile(shp, F32, tag="wm12")
            nc.vector.scalar_tensor_tensor(out=wxl, in0=qlxg, scalar=1.0, in1=pxc,
                                           op0=ALU.add, op1=ALU.subtract)
            nc.vector.scalar_tensor_tensor(out=wyl, in0=qlyg, scalar=1.0, in1=pyc,
                                           op0=ALU.add, op1=ALU.subtract)
            nc.vector.scalar_tensor_tensor(out=wxr, in0=qrxg, scalar=-1.0, in1=pxc,
                                           op0=ALU.mult, op1=ALU.add)
            nc.vector.tensor_scalar_add(wxr, wxr, 1.0)
            nc.vector.scalar_tensor_tensor(out=wyr, in0=qryg, scalar=-1.0, in1=pyc,
                                           op0=ALU.mult, op1=ALU.add)
            nc.vector.tensor_scalar_add(wyr, wyr, 1.0)
            modv = workp.tile(shp, F32, tag="wm13")
            nc.scalar.activation(out=modv, in_=offsig[:, :, 2 * NTAP:3 * NTAP],
                                 func=AF.Sigmoid, bias=0.0, scale=1.0)
            nc.vector.tensor_tensor(out=wxl, in0=wxl, in1=modv, op=ALU.mult)
            nc.vector.tensor_tensor(out=wxr, in0=wxr, in1=modv, op=ALU.mult)
            wA = singles.tile(shp, F32)
            wB = singles.tile(shp, F32)
            wC = singles.tile(shp, F32)
            wD = singles.tile(shp, F32)
            nc.vector.tensor_tensor(out=wA, in0=wxl, in1=wyl, op=ALU.mult)
            nc.vector.tensor_tensor(out=wB, in0=wxl, in1=wyr, op=ALU.mult)
            nc.vector.tensor_tensor(out=wC, in0=wxr, in1=wyl, op=ALU.mult)
            nc.vector.tensor_tensor(out=wD, in0=wxr, in1=wyr, op=ALU.mult)

            with tc.tile_pool(name="psMapA", bufs=1, space="PSUM") as psMap, \
                 tc.tile_pool(name="psCTX", bufs=1, space="PSUM") as psCTX:
                ctx_ps = psCTX.tile([1, 257], F32)
                for m in range(MCH):
                    valid = 128 if m < MCH - 1 else MPIX - 128 * (MCH - 1)
                    tp = psMap.tile([128, 256], BF16, tag="tp", bufs=3)
                    for ch in range(2):
                        nc.tensor.transpose(tp[:valid, ch * 128:(ch + 1) * 128],
                                            bandf[ch][:, m * 128: m * 128 + valid], sb_idb)
                    nc.vector.tensor_copy(mapsb[:valid, m, :], tp[:valid])
                    mk = psMap.tile([128, 1], F32, tag="mk", bufs=3)
                    for ch in range(2):
                        nc.tensor.matmul(mk[:valid], bandf[ch][:, m * 128: m * 128 + valid],
                                         sb_cmw[:, ch:ch + 1],
                                         start=(ch == 0), stop=(ch == 1))
                    e_f = workp.tile([128, 1], F32, tag="e_f", bufs=3)
                    nc.scalar.activation(out=e_f[:valid], in_=mk[:valid], func=AF.Exp,
                                         bias=sb_cmb[:valid], scale=1.0)
                    e_b = workp.tile([128, 1], BF16, tag="e_b", bufs=3)
                    nc.vector.tensor_tensor(out=e_b[:valid], in0=e_f[:valid],
                                            in1=sb_own[:valid, m:m + 1], op=ALU.mult)
                    nc.tensor.matmul(ctx_ps[:, 0:256], e_b[:valid], mapsb[:valid, m, :],
                                     start=(m == 0), stop=(m == MCH - 1))
                    nc.tensor.matmul(ctx_ps[:, 256:257], e_b[:valid], ones_col[:valid],
                                     start=(m == 0), stop=(m == MCH - 1))
                ctx_sb = workp.tile([1, 257], F32, tag="ctxsb")
                nc.vector.tensor_copy(ctx_sb, ctx_ps)
                nc.sync.dma_start(out=bass.AP(tensor=stats, offset=512, ap=[[1, 1], [1, 257]]),
                                  in_=ctx_sb)

                # ----- map2 doubled-row writes (3 DMAs) -----
                nc.sync.dma_start(
                    out=bass.AP(tensor=map2, offset=0, ap=[[512, 128], [512 * 128, MCH], [1, 256]]),
                    in_=mapsb[:, :, :])
                nc.sync.dma_start(
                    out=bass.AP(tensor=map2, offset=256, ap=[[512, 62], [512 * 128, MCH], [1, 256]]),
                    in_=mapsb[66:128, :, :])
                nc.sync.dma_start(
                    out=bass.AP(tensor=map2, offset=62 * 512 + 256,
                                ap=[[512, 66], [512 * 128, MCH - 1], [1, 256]]),
                    in_=mapsb[0:66, 1:MCH, :])

            # ----- gather / diag-weighted corner combine / DCN matmul -----
            y_sb = [singles.tile([128, NPIX], BF16, tag=f"ysb{c_}", name=f"ysb{c_}") for c_ in range(2)]
            s1parts = [singles.tile([128, 4], F32, tag=f"s1p{c_}", name=f"s1p{c_}") for c_ in range(2)]
            sqparts = [singles.tile([128, 2], F32, tag=f"sqp{c_}", name=f"sqp{c_}") for c_ in range(2)]
            scratch = [singles.tile([128, NPIX // 2], BF16, tag=f"scr{c_}", name=f"scr{c_}") for c_ in range(2)]
            map_ap = bass.AP(tensor=map2, offset=0, ap=[[512, MAP2_ROWS - 1], [1, 1024]])
            wk = (wA, wC, wB, wD)  # chunk order: (ql,ql), (ql+1,ql), (ql,ql+1), (ql+1,ql+1)
            with tc.tile_pool(name="psXO", bufs=1, space="PSUM") as psXO, \
                 tc.tile_pool(name="psY", bufs=1, space="PSUM") as psY:
                for g in range(2):
                    yps = [psY.tile([128, 512], F32, tag=f"yps{h}{o}", name=f"yps{h}{o}")
                           for h in range(2) for o in range(2)]
                    for h in range(2):
                        for o in range(2):
                            nc.tensor.matmul(yps[h * 2 + o], sb_dcnb[:, o * 128:(o + 1) * 128],
                                             sb_ones[:, 0:512], start=True, stop=False)
                    for n in range(NTAP):
                        G = gpool.tile([128, 8, 1024], MAPDT, tag="G")
                        blk = g * 16 + n
                        nc.gpsimd.dma_gather(
                            out_ap=G[:, :, :], in_ap=map_ap,
                            idxs_ap=idxw[:, blk * 64:(blk + 1) * 64],
                            num_idxs=1024, num_idxs_reg=1024,
                            elem_size=1024, elem_step=512)
                        for h in range(2):
                            diags = dpool.tile([128, 16, 128], BF16, tag="diag")
                            for tl4 in range(4):
                                t_abs = g * 8 + h * 4 + tl4
                                for k in range(4):
                                    nc.vector.tensor_scalar_mul(diags[:, tl4 * 4 + k], sb_idb,
                                                                wk[k][:, t_abs, n:n + 1])
                            xoc = psXO.tile([128, 2, 512], F32, tag=f"xo{h}", name=f"xo{h}")
                            for tl4 in range(4):
                                for ch in range(2):
                                    for k in range(4):
                                        nc.tensor.matmul(
                                            xoc[:, ch, tl4 * 128:(tl4 + 1) * 128],
                                            G[:, h * 4 + tl4, k * 256 + ch * 128: k * 256 + ch * 128 + 128],
                                            diags[:, tl4 * 4 + k, :],
                                            start=(k == 0), stop=(k == 3))
                            xos = xop.tile([128, 2, 512], FP8 if FP8_DCN else BF16, tag=f"xos{h}")
                            nc.scalar.copy(xos, xoc)
                            if FP8_DCN:
                                for o in range(2):
                                    nc.tensor.matmul(yps[h * 2 + o],
                                                     sb_dcnw[:, :, n, o * 128:(o + 1) * 128],
                                                     xos[:, :, :],
                                                     start=False, stop=(n == NTAP - 1), perf_mode=DR)
                            else:
                                for ch in range(2):
                                    for o in range(2):
                                        nc.tensor.matmul(yps[h * 2 + o],
                                                         sb_dcnw[:, ch, n, o * 128:(o + 1) * 128],
                                                         xos[:, ch],
                                                         start=False, stop=(n == NTAP - 1 and ch == 1))
                    for h in range(2):
                        for o in range(2):
                            # un-permute sigma on the copy out; BN sum rides accum_out
                            dsty = bass.AP(tensor=y_sb[o].tensor,
                                           offset=y_sb[o].offset + (g * 2 + h) * 512,
                                           ap=[y_sb[o].ap[0], [128, 4], [1, 8], [8, 16]])
                            srcy = bass.AP(tensor=yps[h * 2 + o].tensor,
                                           offset=yps[h * 2 + o].offset,
                                           ap=[yps[h * 2 + o].ap[0], [128, 4], [16, 8], [1, 16]])
                            nc.scalar.activation(out=dsty, in_=srcy, func=AF.Copy,
                                                 scale=1.0 / DCN_SCALE,
                                                 accum_out=s1parts[o][:, g * 2 + h: g * 2 + h + 1])
                    for o in range(2):
                        nc.scalar.activation(out=scratch[o], in_=y_sb[o][:, g * 1024:(g + 1) * 1024],
                                             func=AF.Square, accum_out=sqparts[o][:, g:g + 1])
                        nc.sync.dma_start(out=y_out[o][:, g * 1024:(g + 1) * 1024],
                                          in_=y_sb[o][:, g * 1024:(g + 1) * 1024])

            # ----- BN partial sums -----
            stat4 = workp.tile([128, 4], F32, tag="stat4")
            for o in range(2):
                nc.vector.tensor_reduce(stat4[:, o:o + 1], s1parts[o],
                                        axis=mybir.AxisListType.X, op=ALU.add)
                nc.vector.tensor_reduce(stat4[:, 2 + o:3 + o], sqparts[o],
                                        axis=mybir.AxisListType.X, op=ALU.add)
            nc.sync.dma_start(out=bass.AP(tensor=stats, offset=0, ap=[[1, 128], [128, 4]]),
                              in_=stat4)
    nc.compile()
    return nc


def build_phase_b():
    nc = bacc.Bacc("TRN2", target_bir_lowering=False)
    y_in = nc.dram_tensor("y_in", [2, 128, NPIX], BF16, kind="ExternalInput")
    pooled_in = nc.dram_tensor("pooled_in", [2, 128, NPIX], BF16, kind="ExternalInput")
    # packed per-channel params: [bnsum, bnsq, ctxv, bng, bnb, fb, c2b, pad]
    pk = nc.dram_tensor("pk", [2, 128, 8], F32, kind="ExternalInput")
    c1wT = nc.dram_tensor("c1wT", [2, 128, RR], BF16, kind="ExternalInput")
    c1b = nc.dram_tensor("c1b", [RR, 1], F32, kind="ExternalInput")
    lnpk = nc.dram_tensor("lnpk", [2 * RR, 1], F32, kind="ExternalInput")
    c2wT = nc.dram_tensor("c2wT", [RR, C], BF16, kind="ExternalInput")
    fwT = nc.dram_tensor("fwT", [128, 8, 128], BF16, kind="ExternalInput")
    identb = nc.dram_tensor("identb", [128, 128], BF16, kind="ExternalInput")
    identf = nc.dram_tensor("identf", [128, 128], F32, kind="ExternalInput")

    outh = nc.dram_tensor("outh", [2, 128, NPIX], F32, kind="ExternalOutput")

    with tile.TileContext(nc) as tc:
        with tc.tile_pool(name="singles", bufs=1) as singles, \
             tc.tile_pool(name="workp", bufs=2) as workp, \
             tc.tile_pool(name="ps", bufs=1, space="PSUM") as ps, \
             tc.tile_pool(name="psf", bufs=3, space="PSUM") as psf:
            pks = singles.tile([128, 2, 8], F32)
            nc.sync.dma_start(
                out=pks, in_=bass.AP(tensor=pk, offset=0,
                                     ap=[[8, 128], [128 * 8, 2], [1, 8]]))
            ysb = singles.tile([128, 2, NPIX], BF16)
            psb = singles.tile([128, 2, NPIX], BF16)
            for t, d in ((ysb, y_in), (psb, pooled_in)):
                for ch in range(2):
                    nc.sync.dma_start(out=t[:, ch], in_=d[ch])
            sb_c1w = singles.tile([128, 2, RR], BF16)
            for ch in range(2):
                nc.sync.dma_start(out=sb_c1w[:, ch], in_=c1wT[ch])
            sb_c1b = singles.tile([RR, 1], F32)
            nc.sync.dma_start(out=sb_c1b, in_=c1b[:, :])
            sb_lngc = singles.tile([RR, 1], F32)
            nc.sync.dma_start(out=sb_lngc, in_=lnpk[0:RR, :])
            sb_lnbc = singles.tile([RR, 1], F32)
            nc.sync.dma_start(out=sb_lnbc, in_=lnpk[RR:2 * RR, :])
            sb_c2w = singles.tile([RR, C], BF16)
            nc.sync.dma_start(out=sb_c2w, in_=c2wT[:, :])
            sb_fw = singles.tile([128, 8, 128], BF16)
            nc.sync.dma_start(out=sb_fw, in_=fwT[:, :])
            sb_idb = singles.tile([128, 128], BF16)
            nc.sync.dma_start(out=sb_idb, in_=identb[:, :])
            sb_idf = singles.tile([128, 128], F32)
            nc.sync.dma_start(out=sb_idf, in_=identf[:, :])
            epsv = singles.tile([128, 1], F32)
            nc.vector.memset(epsv, EPS)
            ones_rr = singles.tile([RR, 1], BF16)
            nc.vector.memset(ones_rr, 1.0)
            ones_row = singles.tile([1, RR], F32)
            nc.vector.memset(ones_row, 1.0)


            # BN scale/shift
            ybn = [singles.tile([128, NPIX], BF16, tag=f"ybn{c_}", name=f"ybn{c_}") for c_ in range(2)]
            biasF = [workp.tile([128, 1], F32, tag=f"bf{c_}", name=f"biasF{c_}") for c_ in range(2)]
            for ch in range(2):
                mu = workp.tile([128, 1], F32, tag="mu")
                nc.vector.tensor_scalar_mul(mu, pks[:, ch, 0:1], 1.0 / N_TOT)
                s2n = workp.tile([128, 1], F32, tag="s2n")
                nc.vector.tensor_scalar_mul(s2n, pks[:, ch, 1:2], 1.0 / N_TOT)
                negmu = workp.tile([128, 1], F32, tag="negmu")
                nc.vector.tensor_scalar_mul(negmu, mu, -1.0)
                var = workp.tile([128, 1], F32, tag="var")
                nc.vector.scalar_tensor_tensor(out=var, in0=mu, scalar=negmu, in1=s2n,
                                               op0=ALU.mult, op1=ALU.add)
                std = workp.tile([128, 1], F32, tag="std")
                nc.scalar.activation(out=std, in_=var, func=AF.Sqrt, bias=epsv, scale=1.0)
                rstd = workp.tile([128, 1], F32, tag="rstd")
                nc.vector.reciprocal(rstd, std)
                scale = workp.tile([128, 1], F32, tag="scale")
                nc.vector.tensor_tensor(out=scale, in0=pks[:, ch, 3:4], in1=rstd, op=ALU.mult)
                shift = workp.tile([128, 1], F32, tag="shift")
                nc.vector.scalar_tensor_tensor(out=shift, in0=scale, scalar=negmu,
                                               in1=pks[:, ch, 4:5], op0=ALU.mult, op1=ALU.add)
                nc.scalar.activation(out=ybn[ch], in_=ysb[:, ch], func=AF.Relu,
                                     bias=shift, scale=scale)

            # GCNet MLP
            ctxb = workp.tile([128, 2], BF16, tag="ctxb")
            for ch in range(2):
                nc.vector.tensor_copy(ctxb[:, ch:ch + 1], pks[:, ch, 2:3])
            t1p = ps.tile([RR, 1], F32)
            for ch in range(2):
                nc.tensor.matmul(t1p, sb_c1w[:, ch], ctxb[:, ch:ch + 1],
                                 start=(ch == 0), stop=(ch == 1))
            t1s = workp.tile([RR, 1], BF16, tag="t1s")
            t1f = workp.tile([RR, 1], F32, tag="t1f")
            nc.vector.tensor_tensor(out=t1f, in0=t1p, in1=sb_c1b, op=ALU.add)
            nc.vector.tensor_copy(t1s, t1f)
            # sum and sum-of-squares over partitions via PE
            s12 = ps.tile([1, 2], F32, tag="s12")
            nc.tensor.matmul(s12[:, 0:1], t1s, ones_rr, start=True, stop=True)
            nc.tensor.matmul(s12[:, 1:2], t1s, t1s, start=True, stop=True)
            m1 = workp.tile([1, 2], F32, tag="m1")
            nc.vector.tensor_scalar_mul(m1, s12, 1.0 / RR)        # [mean, E[x^2]]
            v1 = workp.tile([1, 1], F32, tag="v1")
            mm = workp.tile([1, 1], F32, tag="mm")
            nc.vector.tensor_tensor(out=mm, in0=m1[:, 0:1], in1=m1[:, 0:1], op=ALU.mult)
            nc.vector.tensor_tensor(out=v1, in0=m1[:, 1:2], in1=mm, op=ALU.subtract)
            nc.scalar.activation(out=v1, in_=v1, func=AF.Sqrt, bias=epsv[0:1], scale=1.0)
            nc.vector.reciprocal(v1, v1)
            nc.vector.tensor_tensor(out=m1[:, 0:1], in0=m1[:, 0:1], in1=v1, op=ALU.mult)
            mbc = ps.tile([RR, 2], F32, tag="mbc")     # broadcast [rstd, mean*rstd]
            nc.tensor.matmul(mbc[:, 0:1], ones_row, v1, start=True, stop=True)
            nc.tensor.matmul(mbc[:, 1:2], ones_row, m1[:, 0:1], start=True, stop=True)
            tn = workp.tile([RR, 1], F32, tag="tn")
            nc.vector.scalar_tensor_tensor(out=tn, in0=t1f, scalar=mbc[:, 0:1],
                                           in1=mbc[:, 1:2], op0=ALU.mult, op1=ALU.subtract)
            nc.vector.scalar_tensor_tensor(out=tn, in0=tn, scalar=sb_lngc, in1=sb_lnbc,
                                           op0=ALU.mult, op1=ALU.add)
            trt = workp.tile([RR, 1], BF16, tag="trt")
            nc.scalar.activation(out=trt, in_=tn, func=AF.Relu, bias=0.0, scale=1.0)
            tvb = [workp.tile([128, 1], BF16, tag=f"tvb{c_}", name=f"tvb{c_}") for c_ in range(2)]
            tvf = [workp.tile([128, 1], F32, tag=f"tvf{c_}", name=f"tvf{c_}") for c_ in range(2)]
            for ch in range(2):
                tp2 = ps.tile([128, 1], F32, tag="tp2")
                nc.tensor.matmul(tp2, sb_c2w[:, ch * 128:(ch + 1) * 128], trt,
                                 start=True, stop=True)
                nc.vector.tensor_tensor(out=tvf[ch], in0=tp2, in1=pks[:, ch, 6:7], op=ALU.add)
                nc.vector.tensor_copy(tvb[ch], tvf[ch])
            # biasF[o] = fb[o] + (fwp_z . tv)[o] - tv[o]   (fusion feeds raw pooled x)
            for o in range(2):
                fzp = ps.tile([128, 1], F32, tag="fzp")
                for ch in range(2):
                    nc.tensor.matmul(fzp, sb_fw[:, (2 + ch) * 2 + o], tvb[ch],
                                     start=(ch == 0), stop=(ch == 1))
                nc.vector.tensor_tensor(out=biasF[o], in0=fzp, in1=pks[:, o, 5:6], op=ALU.add)
                nc.vector.tensor_tensor(out=biasF[o], in0=biasF[o], in1=tvf[o], op=ALU.subtract)

            # fusion
            outsb = [singles.tile([128, NPIX], F32, tag=f"o{c_}", name=f"outsb{c_}") for c_ in range(2)]
            rhs = [ybn[0], ybn[1], None, None]
            for o in range(2):
                for pt in range(4):
                    pf = psf.tile([128, 512], F32, tag="pf")
                    for k in range(4):
                        r = rhs[k][:, pt * 512:(pt + 1) * 512] if k < 2 else \
                            psb[:, k - 2, pt * 512:(pt + 1) * 512]
                        nc.tensor.matmul(pf, sb_fw[:, k * 2 + o], r,
                                         start=(k == 0), stop=(k == 3))
                    nc.scalar.activation(out=outsb[o][:, pt * 512:(pt + 1) * 512], in_=pf,
                                         func=AF.Identity, bias=biasF[o], scale=1.0)
                    nc.sync.dma_start(out=outh[o][:, pt * 512:(pt + 1) * 512],
                                      in_=outsb[o][:, pt * 512:(pt + 1) * 512])
    nc.compile()
    return nc


# ---------------- host side ----------------
_CACHE = {}
EXEC_NS = []


def _run(nc, in_maps):
    if os.environ.get("KERNEL_SIM"):
        from concourse.bass_interp import CoreSim
        outs = []
        for i, im in enumerate(in_maps):
            sim = CoreSim(nc, require_finite=False, require_nnan=False)
            for k, v in im.items():
                sim.tensor(k)[:] = v
            sim.simulate(check_with_hw=False)
            out_allocs = {a.memorylocations[0].name: list(a.tensor_shape)
                          for a in nc.m.functions[0].allocations
                          if getattr(a, "kind", None) == "ExternalOutput"}
            outs.append({k: np.array(sim.mem_tensor(k)).reshape(shp)
                         for k, shp in out_allocs.items()})
            print(f"  sim core {i} done")
        return outs
    res = run_bass_kernel_spmd(nc, in_maps, core_ids=list(range(8)))
    if res.exec_time_ns is not None:
        EXEC_NS.append(res.exec_time_ns)
    return res.results


def _consts():
    if "c" in _CACHE:
        return _CACHE["c"]
    rng3 = np.arange(-1, 2)
    pnx = np.repeat(rng3, 3).astype(np.float32)   # tap n = (dy+1)*3+(dx+1)
    pny = np.tile(rng3, 3).astype(np.float32)
    p = np.arange(128)
    t = np.arange(16)
    s_nat = t[None, :] * 128 + p[:, None]          # [128,16]
    s_sig = t[None, :] * 128 + SIG[p][:, None]
    consts = {}
    for hh in range(2):
        g0 = 1 + 32 * hh
        r_nat = s_nat // 64
        c_nat = s_nat % 64
        r_sig = s_sig // 64
        c_sig = s_sig % 64
        consts[hh] = dict(
            p0xl8=(OWN0 + r_nat[:, :, None] + pnx[None, None, :] - 8.0).astype(np.float32).reshape(128, -1),
            p0yl8=(c_nat[:, :, None] + 1 + pny[None, None, :] - 8.0).astype(np.float32).reshape(128, -1),
            p0xs=(g0 + r_sig[:, :, None] + pnx[None, None, :]).astype(np.float32).reshape(128, -1),
            p0ys=(c_sig[:, :, None] + 1 + pny[None, None, :]).astype(np.float32).reshape(128, -1),
        )
    mp = np.arange(MCH * 128)
    mrow, mcol = mp // HP, mp % HP
    own = ((mrow >= OWN0) & (mrow < OWN0 + OWN) & (mcol >= 1) & (mcol < 65) & (mp < MPIX))
    ownm = own.astype(np.float32).reshape(MCH, 128).T.copy()   # [128, MCH]
    identb = np.eye(128, dtype=BF)
    identf = np.eye(128, dtype=np.float32)
    _CACHE["c"] = (consts, ownm, identb, identf)
    return _CACHE["c"]


def kernel(x, p_w, p_b, m_w, m_b, dcn_w, dcn_b, bn_g, bn_b,
           cm_w, cm_b, c1_w, c1_b, ln_g, ln_b, c2_w, c2_b, f_w, f_b):
    x = np.asarray(x, np.float32)
    consts, ownm, identb, identf = _consts()

    # weights prep
    pm = np.concatenate([np.asarray(p_w), np.asarray(m_w)], 0).astype(np.float32)  # [27,256,3,3]
    pmw = np.zeros((2, 128, NTAP * 32), E4 if FP8_OFF else BF)
    for ch in range(2):
        for n in range(NTAP):
            pmw[ch, :, n * 32:n * 32 + 27] = (pm[:, ch * 128:(ch + 1) * 128, n // 3, n % 3].T
                                              * PMW_SCALE).astype(pmw.dtype)
    pmb_h = (np.concatenate([np.asarray(p_b), np.asarray(m_b)]) * PMW_SCALE).astype(BF)[None, :]
    dw = np.asarray(dcn_w, np.float32).reshape(C, C, NTAP)
    dcnw_h = np.zeros((2, 128, NTAP * C), E4 if FP8_DCN else BF)
    for ch in range(2):
        for n in range(NTAP):
            dcnw_h[ch, :, n * C:(n + 1) * C] = (dw[:, ch * 128:(ch + 1) * 128, n].T
                                                * DCN_SCALE).astype(dcnw_h.dtype)
    dcnb_h = (np.asarray(dcn_b, np.float32) * DCN_SCALE).astype(BF)[None, :]
    cmw_h = np.asarray(cm_w, np.float32).reshape(C).astype(BF).reshape(2, 128)
    cmb_h = np.full((128, 1), float(np.asarray(cm_b).reshape(-1)[0]), np.float32)
    fw2 = np.asarray(f_w, np.float32).reshape(C, 2 * C)
    fwp = fw2.copy()
    fwp[:, C:] += np.eye(C, dtype=np.float32)
    fwT_h = np.zeros((128, 8, 128), BF)
    for kc in range(4):
        for oc in range(2):
            fwT_h[:, kc * 2 + oc, :] = fwp[oc * 128:(oc + 1) * 128, kc * 128:(kc + 1) * 128].T.astype(BF)
    c1w2 = np.asarray(c1_w, np.float32).reshape(RR, C)
    c1wT_h = np.stack([c1w2[:, ch * 128:(ch + 1) * 128].T.astype(BF) for ch in range(2)])
    c2w2 = np.asarray(c2_w, np.float32).reshape(C, RR)
    c2wT_h = c2w2.T.astype(BF)                      # [RR, C]
    c1b_h = np.asarray(c1_b, np.float32).reshape(RR, 1)
    lnpk_h = np.concatenate([np.asarray(ln_g, np.float32).reshape(RR),
                             np.asarray(ln_b, np.float32).reshape(RR)]).reshape(2 * RR, 1)

    xbf = x.astype(BF)
    in_maps_a = []
    for i in range(8):
        s, hh = i // 2, i % 2
        g0 = 1 + 32 * hh
        xin = np.zeros((2, 128, 84, WI), BF)
        for l in range(BAND):
            pr = g0 - 6 + l
            if 0 <= pr < 64:
                xin[:, :, 2 * l:2 * l + 2, :] = xbf[s].reshape(2, 128, HI, WI)[:, :, 2 * pr:2 * pr + 2, :]
        cc = consts[hh]
        in_maps_a.append(dict(
            xin=xin.reshape(2, 128, 84 * WI),
            p0xl8=cc["p0xl8"], p0yl8=cc["p0yl8"], p0xs=cc["p0xs"], p0ys=cc["p0ys"],
            ownm=ownm, cmb=cmb_h, pmw=pmw, pmb=pmb_h, dcnw=dcnw_h, dcnb=dcnb_h,
            cmw=cmw_h, identb=identb, identf=identf,
        ))

    if "nc_a" not in _CACHE:
        _CACHE["nc_a"] = build_phase_a()
        _CACHE["nc_b"] = build_phase_b()
    ra = _run(_CACHE["nc_a"], in_maps_a)

    st = np.stack([ra[i]["stats"][0] for i in range(8)])   # [8, 1032]
    bnsum_tot = st[:, 0:256].sum(0).reshape(2, 128).astype(np.float32)
    bnsq_tot = st[:, 256:512].sum(0).reshape(2, 128).astype(np.float32)
    ctx_all = []
    for s in range(4):
        p1 = st[2 * s, 512:768] + st[2 * s + 1, 512:768]
        z = st[2 * s, 768] + st[2 * s + 1, 768]
        ctx_all.append((p1 / z).reshape(2, 128).astype(np.float32))

    two = lambda v: np.asarray(v, np.float32).reshape(2, 128)
    bng_h, bnb_h, fb_h, c2b_h = two(bn_g), two(bn_b), two(f_b), two(c2_b)

    in_maps_b = []
    for i in range(8):
        s = i // 2
        pk = np.stack([bnsum_tot, bnsq_tot, ctx_all[s], bng_h, bnb_h, fb_h, c2b_h,
                       np.zeros((2, 128), np.float32)], axis=2)   # [2, 128, 8]
        in_maps_b.append(dict(
            y_in=ra[i]["y_out"], pooled_in=ra[i]["pooled_out"],
            pk=pk.astype(np.float32),
            c1wT=c1wT_h, c1b=c1b_h, lnpk=lnpk_h, c2wT=c2wT_h,
            fwT=fwT_h, identb=identb, identf=identf,
        ))
    rb = _run(_CACHE["nc_b"], in_maps_b)

    out = np.zeros((B, C, H, W), np.float32)
    for i in range(8):
        s, hh = i // 2, i % 2
        oh = rb[i]["outh"].reshape(2, 128, OWN, W)
        out[s, 0:128, hh * OWN:(hh + 1) * OWN, :] = oh[0]
        out[s, 128:256, hh * OWN:(hh + 1) * OWN, :] = oh[1]
    return out
